# revision 53
# baseline (speedup 1.0000x reference)
"""Copy-enhanced CodeT5 head (histogram/scatter blend) on 8 TRN2 NeuronCores.

Strategy: data-parallel over (batch, T/2) -> 8 shards of 128 decoder rows.
Each core, for its [128, V] output block:
  A_sum    = sum_h cross_attn[h]                       (DVE adds)
  p_gen    = sigmoid((A_sum @ (enc @ W1))/H + dec.W2 + b)   (PE + DVE dots + ACT)
  exp, Z   = exp(logits) streamed, row-sums via ACT accum   (pass 1)
  P_copy   = scatter-add of (1-p_gen)/H * (A_sum @ Sel) into a bf16
             pair-packed accumulator via gpsimd scatter_add; duplicate
             source ids are pre-combined with a selection-matrix matmul
             and non-first occurrences are redirected to a dump slot
             (the hardware scatter pipeline does not accumulate racing
             duplicate indices).
  out      = exp * (p_gen/Z) + P_copy                  (one fused DVE op, pass 2)

No collectives needed: every core owns a disjoint output block.
"""
import sys

sys.path.insert(0, "/opt/trn_rl_repo")

import numpy as np

import concourse.bass as bass  # noqa: F401  (registers engine classes)
import concourse.mybir as mybir
from concourse import bacc, bass_utils, library_config
from concourse.tile import TileContext
from concourse.masks import make_identity

B, S, T, D, H, V = 4, 512, 256, 1024, 16, 32105
P = 128
NCORES = 8
NPAIR = V // 2 + 2          # 16054 pair slots; pairs 0..16052 hold vocab, 16053 = dump
DUMP = NPAIR - 1
VTILE = 1024
NT = (V + VTILE - 1) // VTILE
NEARLY = 8
LIB_PRELOAD = True
IDX_DRAM = True

AluOp = mybir.AluOpType
Act = mybir.ActivationFunctionType
f32 = mybir.dt.float32
bf16 = mybir.dt.bfloat16
i32 = mybir.dt.int32
i16 = mybir.dt.int16


def _body(tc, ids_d, logits_d, enc_d, dec_d, xattn_d, wgw_d, wgb_d, scr_d,
          out_d):
    nc = tc.nc
    with tc.tile_pool(name="fix", bufs=1) as fix, \
         tc.tile_pool(name="work", bufs=4) as work, \
         tc.tile_pool(name="lpool", bufs=6) as lpool, \
         tc.tile_pool(name="psum", bufs=1, space="PSUM") as psum:

        # ---- head sum via SWDGE accumulating DMAs: the compute-DMA adds
        # each head into A on the fly. No SBUF for 16 head tiles, no DVE
        # adds, and the Q0 ring runs concurrently with the Q1 input loads.
        A = fix.tile([P, S], f32)
        for h in range(H):
            nc.gpsimd.dma_start(out=A[:], in_=xattn_d[h],
                                accum_op=(AluOp.bypass if h == 0
                                          else AluOp.add))

        # preload the gpsimd ucode library that scatter_add needs; the ~13us
        # load runs under the input-DMA phase instead of gating the scatter
        if LIB_PRELOAD:
            nc.gpsimd.load_library(library_config.mlp)

        # ---- persistent tiles ----
        exp_store = fix.tile([P, V], bf16)
        pcopy = fix.tile([P, NPAIR, 2], bf16)
        # zero the accumulator on ACT (otherwise idle before the exps);
        # emitted first so the DVE prologue chain stays unblocked
        nc.scalar.memzero(pcopy[:])

        ident = fix.tile([P, P], f32)
        make_identity(nc, ident[:])

        # ---- ALL input DMAs up front, COALESCED (the sync sequencer takes
        # ~0.6us per trigger: 25 triggers used to delay the ltile stream by
        # 21us). ids first so the pair/Sel chain starts immediately. No
        # trigger here may block on a consumer, or it head-of-line-stalls
        # the sync sequencer and starves everything behind it.
        ids_bc_i = work.tile([P, S], i32, tag="pair", bufs=1)
        nc.sync.dma_start(out=ids_bc_i[:], in_=ids_d[None, :].to_broadcast((P, S)))
        ids_col_i = fix.tile([P, 4], i32)
        nc.sync.dma_start(out=ids_col_i[:], in_=ids_d.rearrange("(c p) -> p c", p=P))
        wg = work.tile([P, 2 * D], f32, tag="wgt", bufs=1)
        nc.sync.dma_start(out=wg[:], in_=wgw_d[0:1, :].to_broadcast((P, 2 * D)))
        wb_bc = fix.tile([P, 1], f32)
        nc.sync.dma_start(out=wb_bc[:], in_=wgb_d[None, :].to_broadcast((P, 1)))
        dec_t = work.tile([P, D], f32, tag="dec", bufs=1)
        nc.sync.dma_start(out=dec_t[:], in_=dec_d[:])
        enc_g = work.tile([P, 4, D], f32, tag="enc", bufs=1)
        nc.sync.dma_start(out=enc_g[:], in_=enc_d.rearrange("(c p) d -> p c d", p=P))

        # ---- pass-1 load stream issued NOW (before any compute-dependent
        # DMAs can stall the sync sequencer); first few exps too, so the
        # p_gen exp below lands between exp11 and exp12 on the in-order ACT
        zparts = fix.tile([P, NT], f32)
        ltiles = []
        for k in range(NT):
            ltile = lpool.tile([P, VTILE], f32, tag="lt", name=f"lt{k}")
            w_k = min(VTILE, V - k * VTILE)
            nc.sync.dma_start(out=ltile[:, :w_k],
                              in_=logits_d[:, k * VTILE:k * VTILE + w_k])
            ltiles.append(ltile)
        # exps WITHOUT the ACT accumulator read (which costs 279ns/tile and
        # made ACT the load-stream pacer); row-sums go to DVE instead
        for k in range(NEARLY):
            off = k * VTILE
            w_k = min(VTILE, V - off)
            nc.scalar.activation(out=exp_store[:, off:off + w_k],
                                 in_=ltiles[k][:, :w_k], func=Act.Exp)
            nc.vector.tensor_reduce(out=zparts[:, k:k + 1],
                                    in_=exp_store[:, off:off + w_k],
                                    axis=mybir.AxisListType.X, op=AluOp.add)

        # ---- A^T via PE transposes (bf16: feeds bf16 matmuls/scatter) ----
        A_T = fix.tile([P, 4, P], bf16)
        for kk in range(4):
            tps = psum.tile([P, P], f32, tag="tps", bufs=2, name=f"tps{kk}")
            nc.tensor.transpose(tps[:], A[:, kk * P:(kk + 1) * P], ident[:])
            nc.vector.tensor_copy(out=A_T[:, kk, :], in_=tps[:])

        # ---- p_gen (emit early: its sigmoid must precede the exps on ACT) ----
        u_col = fix.tile([P, 4], f32)
        for kk in range(4):
            junk = work.tile([P, D], bf16, tag="jnk", name=f"junk{kk}", bufs=1)
            nc.vector.scalar_tensor_tensor(out=junk[:], in0=enc_g[:, kk, :],
                                           scalar=1.0,
                                           in1=wg[:, 0:D], op0=AluOp.mult,
                                           op1=AluOp.mult,
                                           accum_out=u_col[:, kk:kk + 1])
        u_colb = fix.tile([P, 4], bf16)
        nc.vector.tensor_copy(out=u_colb[:], in_=u_col[:])
        plin1_ps = psum.tile([P, 1], f32, tag="plin")
        for kk in range(4):
            nc.tensor.matmul(plin1_ps[:], A_T[:, kk, :], u_colb[:, kk:kk + 1],
                             start=(kk == 0), stop=(kk == 3))
        p_lin2 = fix.tile([P, 1], f32)
        junk2 = work.tile([P, D], bf16, tag="jnk", bufs=1)
        nc.vector.scalar_tensor_tensor(out=junk2[:], in0=dec_t[:], scalar=1.0,
                                       in1=wg[:, D:2 * D], op0=AluOp.mult,
                                       op1=AluOp.mult, accum_out=p_lin2[:])
        p_lin2b = fix.tile([P, 1], f32)
        nc.vector.tensor_add(out=p_lin2b[:], in0=p_lin2[:], in1=wb_bc[:])
        # sigmoid via the already-loaded exp table (avoids two mid-stream
        # ACT table swaps): p_gen = e/(1+e), e = exp(plin1/H + p_lin2b)
        e_sig = fix.tile([P, 1], f32)
        nc.scalar.activation(out=e_sig[:], in_=plin1_ps[:], func=Act.Exp,
                             bias=p_lin2b[:], scale=1.0 / H)
        e1 = fix.tile([P, 1], f32)
        nc.vector.tensor_scalar(e1[:], e_sig[:], 1.0, None, AluOp.add)
        e1r = fix.tile([P, 1], f32)
        nc.vector.reciprocal(out=e1r[:], in_=e1[:])
        p_gen = fix.tile([P, 1], f32)
        nc.vector.tensor_mul(out=p_gen[:], in0=e_sig[:], in1=e1r[:])
        s1 = fix.tile([P, 1], f32)
        nc.vector.tensor_scalar(s1[:], p_gen[:], -1.0 / H, 1.0 / H,
                                AluOp.mult, AluOp.add)

        # ---- pair-level selection matrix + per-lane combine (int compares,
        # bf16 masks: halves SBUF vs the f32 originals) ----
        # shift the ids broadcast to pair ids in place, then convert to f32
        nc.vector.tensor_scalar(ids_bc_i[:], ids_bc_i[:], 1, None,
                                AluOp.arith_shift_right)
        pair_f = work.tile([P, S], f32, tag="pair2", bufs=1)
        nc.vector.tensor_copy(out=pair_f[:], in_=ids_bc_i[:])
        parity_ci = fix.tile([P, 4], i32)
        nc.vector.tensor_scalar(parity_ci[:], ids_col_i[:], 1, None,
                                AluOp.bitwise_and)
        parity_colf = fix.tile([P, 4], f32)
        nc.vector.tensor_copy(out=parity_colf[:], in_=parity_ci[:])
        nc.vector.tensor_scalar(ids_col_i[:], ids_col_i[:], 1, None,
                                AluOp.arith_shift_right)
        pair_colf = fix.tile([P, 4], f32)
        nc.vector.tensor_copy(out=pair_colf[:], in_=ids_col_i[:])
        par_is = fix.tile([P, 4, 2], f32)
        nc.vector.tensor_scalar(par_is[:, :, 0], parity_colf[:], 0.0, None,
                                AluOp.is_equal)
        nc.vector.tensor_scalar(par_is[:, :, 1], parity_colf[:], 1.0, None,
                                AluOp.is_equal)
        Sel = fix.tile([P, 4, S], bf16)
        for kk in range(4):
            nc.vector.tensor_scalar(Sel[:, kk, :], pair_f[:],
                                    pair_colf[:, kk:kk + 1], None, AluOp.is_equal)
        m2 = fix.tile([P, S], bf16)
        comb_e = psum.tile([P, S], f32, tag="combe")
        comb_o = psum.tile([P, S], f32, tag="combo")
        for lane, comb_ps_l in ((0, comb_e), (1, comb_o)):
            for kk in range(4):
                nc.vector.tensor_scalar(m2[:], Sel[:, kk, :],
                                        par_is[:, kk:kk + 1, lane], None, AluOp.mult)
                nc.tensor.matmul(comb_ps_l[:], A_T[:, kk, :], m2[:],
                                 start=(kk == 0), stop=(kk == 3))
        # lower-triangular mask (strictly s' < s), in place; Sel becomes LSel
        for kk in range(4):
            nc.gpsimd.affine_select(
                out=Sel[:, kk, :], in_=Sel[:, kk, :],
                pattern=[[1, S]], compare_op=AluOp.is_ge, fill=0.0,
                base=-(kk * P) - 1, channel_multiplier=-1,
            )
        ones_t = fix.tile([P, 1], bf16)
        nc.vector.memset(ones_t[:], 1.0)
        dup_ps = psum.tile([1, S], f32, tag="dup")
        for kk in range(4):
            nc.tensor.matmul(dup_ps[:], ones_t[:], Sel[:, kk, :],
                             start=(kk == 0), stop=(kk == 3))
        first_occ = fix.tile([1, S], f32)
        nc.vector.tensor_scalar(first_occ[:], dup_ps[:], 0.0, None, AluOp.is_equal)

        # ---- scatter index row: first pair-occurrence -> pair slot, else dump ----
        d1 = fix.tile([1, S], f32)
        nc.vector.tensor_scalar(d1[:], pair_f[0:1, :], -float(DUMP), None,
                                AluOp.add)
        nc.vector.scalar_tensor_tensor(out=d1[:], in0=d1[:], scalar=1.0,
                                       in1=first_occ[:], op0=AluOp.mult,
                                       op1=AluOp.mult)
        nc.vector.tensor_scalar(d1[:], d1[:], float(DUMP), None, AluOp.add)
        idxs_i = fix.tile([1, S], i16)
        nc.vector.tensor_copy(out=idxs_i[:], in_=d1[:])
        # distribute [1, 512] -> [128, 32] in CHUNKED layout: tile[p, i] =
        # row[p*32 + i]; list position j maps to source column
        # sigma(j) = (j % 16)*32 + j // 16 (adds written sigma-permuted below)
        idxs_all = fix.tile([P, 32], i16)
        # SWDGE (gpsimd) so these never stall the sync sequencer's load
        # stream; a DRAM round-trip beats SBUF hop-trees: DRAM APs can
        # broadcast, so the whole distribute is 2 triggers instead of 23
        if IDX_DRAM:
            # scalar (ACT) HWDGE ring: lower trigger latency than SWDGE
            # hops and keeps gpsimd free for the scatter
            nc.scalar.dma_start(out=scr_d, in_=idxs_i[0:1, :])
            nc.scalar.dma_start(
                out=idxs_all[:],
                in_=scr_d.rearrange("a (c i) -> (a c) i", c=16)[None, :, :]
                    .to_broadcast((8, 16, 32)))
        else:
            nc.gpsimd.dma_start(out=idxs_all[0:16, :], in_=idxs_i[0:1, :])
            nc.gpsimd.dma_start(out=idxs_all[16:32, :], in_=idxs_all[0:16, :])
            nc.gpsimd.dma_start(out=idxs_all[32:64, :], in_=idxs_all[0:32, :])
            nc.gpsimd.dma_start(out=idxs_all[64:128, :], in_=idxs_all[0:64, :])

        # ---- scatter adds: pair-packed, both lanes per entry, sigma-permuted
        # (reuses the "pair" buffer: ids_bc_i is dead once pair_f exists)
        add_pairs = work.tile([P, S, 2], bf16, tag="pair", bufs=1)
        add_v = add_pairs[:].rearrange("c (i p) d -> c p i d", p=16)
        nc.vector.tensor_scalar(add_v[:, :, :, 0],
                                comb_e[:].rearrange("c (p i) -> c p i", p=16),
                                s1[:], None, AluOp.mult)
        nc.vector.tensor_scalar(add_v[:, :, :, 1],
                                comb_o[:].rearrange("c (p i) -> c p i", p=16),
                                s1[:], None, AluOp.mult)
        nc.gpsimd.scatter_add(in_ap=pcopy[:], idxs_ap=idxs_all[:],
                              add_ap=add_pairs[:], channels=P, num_elems=NPAIR,
                              d=2, num_idxs=S)

        # ---- pass 1 tail: remaining exps (loads already in flight) ----
        for k in range(NEARLY, NT):
            off = k * VTILE
            w_k = min(VTILE, V - off)
            nc.scalar.activation(out=exp_store[:, off:off + w_k],
                                 in_=ltiles[k][:, :w_k], func=Act.Exp)
            nc.vector.tensor_reduce(out=zparts[:, k:k + 1],
                                    in_=exp_store[:, off:off + w_k],
                                    axis=mybir.AxisListType.X, op=AluOp.add)

        # ---- softmax scale ----
        Z = fix.tile([P, 1], f32)
        nc.vector.tensor_reduce(out=Z[:], in_=zparts[:], axis=mybir.AxisListType.X,
                                op=AluOp.add)
        invZ = fix.tile([P, 1], f32)
        nc.vector.reciprocal(out=invZ[:], in_=Z[:])
        s0 = fix.tile([P, 1], f32)
        nc.vector.tensor_mul(out=s0[:], in0=p_gen[:], in1=invZ[:])

        # ---- pass 2: fused all-bf16 blend IN PLACE over exp_store, then
        # cast-on-store straight from it. Regions are disjoint, so no
        # rotation buffers and the SWDGE queue can hold all 32 transfers.
        pcopy_flat = pcopy[:].rearrange("p a b -> p (a b)")
        # tail (smallest) tile first so the final store transfer is full-size
        for k in [NT - 1] + list(range(NT - 1)):
            off = k * VTILE
            w_k = min(VTILE, V - off)
            nc.vector.scalar_tensor_tensor(
                out=exp_store[:, off:off + w_k],
                in0=exp_store[:, off:off + w_k], scalar=s0[:],
                in1=pcopy_flat[:, off:off + w_k], op0=AluOp.mult, op1=AluOp.add)
            # SWDGE casts bf16 -> f32 on the way out
            nc.gpsimd.dma_start(out=out_d[:, off:off + w_k],
                                in_=exp_store[:, off:off + w_k])


_CACHE = {}


def _get_graph():
    if "nc" in _CACHE:
        return _CACHE["nc"]
    nc = bacc.Bacc("TRN2", target_bir_lowering=False, debug=False,
                   num_devices=NCORES)
    ids_d = nc.dram_tensor("ids", [S], i32, kind="ExternalInput").ap()
    logits_d = nc.dram_tensor("logits", [P, V], f32, kind="ExternalInput").ap()
    enc_d = nc.dram_tensor("enc", [S, D], f32, kind="ExternalInput").ap()
    dec_d = nc.dram_tensor("dec", [P, D], f32, kind="ExternalInput").ap()
    xattn_d = nc.dram_tensor("xattn", [H, P, S], f32, kind="ExternalInput").ap()
    wgw_d = nc.dram_tensor("wgw", [1, 2 * D], f32, kind="ExternalInput").ap()
    wgb_d = nc.dram_tensor("wgb", [1], f32, kind="ExternalInput").ap()
    scr_d = nc.dram_tensor("idx_scratch", [1, S], i16, kind="ExternalOutput").ap()
    out_d = nc.dram_tensor("out", [P, V], f32, kind="ExternalOutput").ap()
    with TileContext(nc) as tc:
        _body(tc, ids_d, logits_d, enc_d, dec_d, xattn_d, wgw_d, wgb_d, scr_d,
              out_d)
    nc.compile()
    _CACHE["nc"] = nc
    return nc


def _shard(inputs):
    ids = np.asarray(inputs["input_ids"])
    logits = np.asarray(inputs["logits"], dtype=np.float32)
    enc = np.asarray(inputs["encoder_hidden_states"], dtype=np.float32)
    dec = np.asarray(inputs["decoder_hidden_states"], dtype=np.float32)
    xattn = np.asarray(inputs["cross_attentions"], dtype=np.float32)
    wgw = np.asarray(inputs["W_gen_w"], dtype=np.float32)
    wgb = np.asarray(inputs["W_gen_b"], dtype=np.float32)
    in_maps = []
    for c in range(NCORES):
        b, th = c // 2, c % 2
        t0 = th * P
        in_maps.append({
            "ids": np.ascontiguousarray(ids[b]).astype(np.int32),
            "logits": np.ascontiguousarray(logits[b, t0:t0 + P, :]),
            "enc": np.ascontiguousarray(enc[b]),
            "dec": np.ascontiguousarray(dec[b, t0:t0 + P, :]),
            "xattn": np.ascontiguousarray(xattn[b, :, t0:t0 + P, :]),
            "wgw": wgw,
            "wgb": wgb,
        })
    return in_maps


def run(inputs, trace=False):
    nc = _get_graph()
    in_maps = _shard(inputs)
    res = bass_utils.run_bass_kernel_spmd(nc, in_maps,
                                          core_ids=list(range(NCORES)),
                                          trace=trace)
    out = np.empty((B, T, V), np.float32)
    for c in range(NCORES):
        b, th = c // 2, c % 2
        out[b, th * P:(th + 1) * P, :] = res.results[c]["out"]  # [P, V]
    return out, res


def kernel(**inputs):
    out, _ = run(inputs, trace=False)
    return out



# revision 55
# speedup vs baseline: 1.1960x; 1.1960x over previous
"""Copy-enhanced CodeT5 head (histogram/scatter blend) on 8 TRN2 NeuronCores.

Strategy: data-parallel over (batch, T/2) -> 8 shards of 128 decoder rows.
Each core, for its [128, V] output block:
  A_sum    = sum_h cross_attn[h]                       (DVE adds)
  p_gen    = sigmoid((A_sum @ (enc @ W1))/H + dec.W2 + b)   (PE + DVE dots + ACT)
  exp, Z   = exp(logits) streamed, row-sums via ACT accum   (pass 1)
  P_copy   = scatter-add of (1-p_gen)/H * (A_sum @ Sel) into a bf16
             pair-packed accumulator via gpsimd scatter_add; duplicate
             source ids are pre-combined with a selection-matrix matmul
             and non-first occurrences are redirected to a dump slot
             (the hardware scatter pipeline does not accumulate racing
             duplicate indices).
  out      = exp * (p_gen/Z) + P_copy                  (one fused DVE op, pass 2)

No collectives needed: every core owns a disjoint output block.
"""
import sys

sys.path.insert(0, "/opt/trn_rl_repo")

import numpy as np

import concourse.bass as bass  # noqa: F401  (registers engine classes)
import concourse.mybir as mybir
from concourse import bacc, bass_utils, library_config
from concourse.tile import TileContext
from concourse.masks import make_identity

B, S, T, D, H, V = 4, 512, 256, 1024, 16, 32105
P = 128
NCORES = 8
NPAIR = V // 2 + 2          # 16054 pair slots; pairs 0..16052 hold vocab, 16053 = dump
DUMP = NPAIR - 1
VTILE = 1024
NT = (V + VTILE - 1) // VTILE
NEARLY = 8
LIB_PRELOAD = True
IDX_DRAM = True

AluOp = mybir.AluOpType
Act = mybir.ActivationFunctionType
f32 = mybir.dt.float32
bf16 = mybir.dt.bfloat16
i32 = mybir.dt.int32
i16 = mybir.dt.int16


def _body(tc, ids_d, logits_d, enc_d, dec_d, xattn_d, wgw_d, wgb_d, scr_d,
          out_d):
    nc = tc.nc
    with tc.tile_pool(name="fix", bufs=1) as fix, \
         tc.tile_pool(name="work", bufs=4) as work, \
         tc.tile_pool(name="lpool", bufs=6) as lpool, \
         tc.tile_pool(name="psum", bufs=1, space="PSUM") as psum:

        # ---- head sum via SWDGE accumulating DMAs: the compute-DMA adds
        # heads into 4 accumulators on the fly. Four round-robin chains so
        # each chain's WAW semaphore round-trip hides behind the other
        # three; a single chain serializes at ~6us per hop.
        A = fix.tile([P, S], f32)
        accs = [fix.tile([P, S], f32, name=f"acc{c}") for c in range(4)]
        for h in range(H):
            c = h % 4
            nc.gpsimd.dma_start(out=accs[c][:], in_=xattn_d[h],
                                accum_op=(AluOp.bypass if h < 4
                                          else AluOp.add))

        # preload the gpsimd ucode library that scatter_add needs; the ~13us
        # load runs under the input-DMA phase instead of gating the scatter
        if LIB_PRELOAD:
            nc.gpsimd.load_library(library_config.mlp)

        # ---- persistent tiles ----
        exp_store = fix.tile([P, V], bf16)
        pcopy = fix.tile([P, NPAIR, 2], bf16)
        # zero the accumulator on ACT (otherwise idle before the exps);
        # emitted first so the DVE prologue chain stays unblocked
        nc.scalar.memzero(pcopy[:])

        ident = fix.tile([P, P], f32)
        make_identity(nc, ident[:])

        # ---- ALL input DMAs up front, COALESCED (the sync sequencer takes
        # ~0.6us per trigger: 25 triggers used to delay the ltile stream by
        # 21us). ids first so the pair/Sel chain starts immediately. No
        # trigger here may block on a consumer, or it head-of-line-stalls
        # the sync sequencer and starves everything behind it.
        ids_bc_i = work.tile([P, S], i32, tag="pair", bufs=1)
        nc.sync.dma_start(out=ids_bc_i[:], in_=ids_d[None, :].to_broadcast((P, S)))
        ids_col_i = fix.tile([P, 4], i32)
        nc.sync.dma_start(out=ids_col_i[:], in_=ids_d.rearrange("(c p) -> p c", p=P))
        wg = work.tile([P, 2 * D], f32, tag="wgt", bufs=1)
        nc.sync.dma_start(out=wg[:], in_=wgw_d[0:1, :].to_broadcast((P, 2 * D)))
        wb_bc = fix.tile([P, 1], f32)
        nc.sync.dma_start(out=wb_bc[:], in_=wgb_d[None, :].to_broadcast((P, 1)))
        dec_t = work.tile([P, D], f32, tag="dec", bufs=1)
        nc.sync.dma_start(out=dec_t[:], in_=dec_d[:])
        enc_g = work.tile([P, 4, D], f32, tag="enc", bufs=1)
        nc.sync.dma_start(out=enc_g[:], in_=enc_d.rearrange("(c p) d -> p c d", p=P))

        # ---- pass-1 load stream issued NOW (before any compute-dependent
        # DMAs can stall the sync sequencer); first few exps too, so the
        # p_gen exp below lands between exp11 and exp12 on the in-order ACT
        zparts = fix.tile([P, NT], f32)
        ltiles = []
        for k in range(NT):
            ltile = lpool.tile([P, VTILE], f32, tag="lt", name=f"lt{k}")
            w_k = min(VTILE, V - k * VTILE)
            nc.sync.dma_start(out=ltile[:, :w_k],
                              in_=logits_d[:, k * VTILE:k * VTILE + w_k])
            ltiles.append(ltile)
        # exps WITHOUT the ACT accumulator read (which costs 279ns/tile and
        # made ACT the load-stream pacer); row-sums go to DVE instead
        for k in range(NEARLY):
            off = k * VTILE
            w_k = min(VTILE, V - off)
            nc.scalar.activation(out=exp_store[:, off:off + w_k],
                                 in_=ltiles[k][:, :w_k], func=Act.Exp)
            nc.vector.tensor_reduce(out=zparts[:, k:k + 1],
                                    in_=exp_store[:, off:off + w_k],
                                    axis=mybir.AxisListType.X, op=AluOp.add)

        # combine the 4 head-chain accumulators
        nc.vector.tensor_add(out=accs[0][:], in0=accs[0][:], in1=accs[1][:])
        nc.vector.tensor_add(out=accs[2][:], in0=accs[2][:], in1=accs[3][:])
        nc.vector.tensor_add(out=A[:], in0=accs[0][:], in1=accs[2][:])

        # ---- A^T via PE transposes (bf16: feeds bf16 matmuls/scatter) ----
        A_T = fix.tile([P, 4, P], bf16)
        for kk in range(4):
            tps = psum.tile([P, P], f32, tag="tps", bufs=2, name=f"tps{kk}")
            nc.tensor.transpose(tps[:], A[:, kk * P:(kk + 1) * P], ident[:])
            nc.vector.tensor_copy(out=A_T[:, kk, :], in_=tps[:])

        # ---- p_gen (emit early: its sigmoid must precede the exps on ACT) ----
        u_col = fix.tile([P, 4], f32)
        for kk in range(4):
            junk = work.tile([P, D], bf16, tag="jnk", name=f"junk{kk}", bufs=1)
            nc.vector.scalar_tensor_tensor(out=junk[:], in0=enc_g[:, kk, :],
                                           scalar=1.0,
                                           in1=wg[:, 0:D], op0=AluOp.mult,
                                           op1=AluOp.mult,
                                           accum_out=u_col[:, kk:kk + 1])
        u_colb = fix.tile([P, 4], bf16)
        nc.vector.tensor_copy(out=u_colb[:], in_=u_col[:])
        plin1_ps = psum.tile([P, 1], f32, tag="plin")
        for kk in range(4):
            nc.tensor.matmul(plin1_ps[:], A_T[:, kk, :], u_colb[:, kk:kk + 1],
                             start=(kk == 0), stop=(kk == 3))
        p_lin2 = fix.tile([P, 1], f32)
        junk2 = work.tile([P, D], bf16, tag="jnk", bufs=1)
        nc.vector.scalar_tensor_tensor(out=junk2[:], in0=dec_t[:], scalar=1.0,
                                       in1=wg[:, D:2 * D], op0=AluOp.mult,
                                       op1=AluOp.mult, accum_out=p_lin2[:])
        p_lin2b = fix.tile([P, 1], f32)
        nc.vector.tensor_add(out=p_lin2b[:], in0=p_lin2[:], in1=wb_bc[:])
        # sigmoid via the already-loaded exp table (avoids two mid-stream
        # ACT table swaps): p_gen = e/(1+e), e = exp(plin1/H + p_lin2b)
        e_sig = fix.tile([P, 1], f32)
        nc.scalar.activation(out=e_sig[:], in_=plin1_ps[:], func=Act.Exp,
                             bias=p_lin2b[:], scale=1.0 / H)
        e1 = fix.tile([P, 1], f32)
        nc.vector.tensor_scalar(e1[:], e_sig[:], 1.0, None, AluOp.add)
        e1r = fix.tile([P, 1], f32)
        nc.vector.reciprocal(out=e1r[:], in_=e1[:])
        p_gen = fix.tile([P, 1], f32)
        nc.vector.tensor_mul(out=p_gen[:], in0=e_sig[:], in1=e1r[:])
        s1 = fix.tile([P, 1], f32)
        nc.vector.tensor_scalar(s1[:], p_gen[:], -1.0 / H, 1.0 / H,
                                AluOp.mult, AluOp.add)

        # ---- pair-level selection matrix + per-lane combine (int compares,
        # bf16 masks: halves SBUF vs the f32 originals) ----
        # shift the ids broadcast to pair ids in place, then convert to f32
        nc.vector.tensor_scalar(ids_bc_i[:], ids_bc_i[:], 1, None,
                                AluOp.arith_shift_right)
        pair_f = work.tile([P, S], f32, tag="pair2", bufs=1)
        nc.vector.tensor_copy(out=pair_f[:], in_=ids_bc_i[:])
        parity_ci = fix.tile([P, 4], i32)
        nc.vector.tensor_scalar(parity_ci[:], ids_col_i[:], 1, None,
                                AluOp.bitwise_and)
        parity_colf = fix.tile([P, 4], f32)
        nc.vector.tensor_copy(out=parity_colf[:], in_=parity_ci[:])
        nc.vector.tensor_scalar(ids_col_i[:], ids_col_i[:], 1, None,
                                AluOp.arith_shift_right)
        pair_colf = fix.tile([P, 4], f32)
        nc.vector.tensor_copy(out=pair_colf[:], in_=ids_col_i[:])
        par_is = fix.tile([P, 4, 2], f32)
        nc.vector.tensor_scalar(par_is[:, :, 0], parity_colf[:], 0.0, None,
                                AluOp.is_equal)
        nc.vector.tensor_scalar(par_is[:, :, 1], parity_colf[:], 1.0, None,
                                AluOp.is_equal)
        Sel = fix.tile([P, 4, S], bf16)
        for kk in range(4):
            nc.vector.tensor_scalar(Sel[:, kk, :], pair_f[:],
                                    pair_colf[:, kk:kk + 1], None, AluOp.is_equal)
        m2 = fix.tile([P, S], bf16)
        comb_e = psum.tile([P, S], f32, tag="combe")
        comb_o = psum.tile([P, S], f32, tag="combo")
        for lane, comb_ps_l in ((0, comb_e), (1, comb_o)):
            for kk in range(4):
                nc.vector.tensor_scalar(m2[:], Sel[:, kk, :],
                                        par_is[:, kk:kk + 1, lane], None, AluOp.mult)
                nc.tensor.matmul(comb_ps_l[:], A_T[:, kk, :], m2[:],
                                 start=(kk == 0), stop=(kk == 3))
        # lower-triangular mask (strictly s' < s), in place; Sel becomes LSel
        for kk in range(4):
            nc.gpsimd.affine_select(
                out=Sel[:, kk, :], in_=Sel[:, kk, :],
                pattern=[[1, S]], compare_op=AluOp.is_ge, fill=0.0,
                base=-(kk * P) - 1, channel_multiplier=-1,
            )
        ones_t = fix.tile([P, 1], bf16)
        nc.vector.memset(ones_t[:], 1.0)
        dup_ps = psum.tile([1, S], f32, tag="dup")
        for kk in range(4):
            nc.tensor.matmul(dup_ps[:], ones_t[:], Sel[:, kk, :],
                             start=(kk == 0), stop=(kk == 3))
        first_occ = fix.tile([1, S], f32)
        nc.vector.tensor_scalar(first_occ[:], dup_ps[:], 0.0, None, AluOp.is_equal)

        # ---- scatter index row: first pair-occurrence -> pair slot, else dump ----
        d1 = fix.tile([1, S], f32)
        nc.vector.tensor_scalar(d1[:], pair_f[0:1, :], -float(DUMP), None,
                                AluOp.add)
        nc.vector.scalar_tensor_tensor(out=d1[:], in0=d1[:], scalar=1.0,
                                       in1=first_occ[:], op0=AluOp.mult,
                                       op1=AluOp.mult)
        nc.vector.tensor_scalar(d1[:], d1[:], float(DUMP), None, AluOp.add)
        idxs_i = fix.tile([1, S], i16)
        nc.vector.tensor_copy(out=idxs_i[:], in_=d1[:])
        # distribute [1, 512] -> [128, 32] in CHUNKED layout: tile[p, i] =
        # row[p*32 + i]; list position j maps to source column
        # sigma(j) = (j % 16)*32 + j // 16 (adds written sigma-permuted below)
        idxs_all = fix.tile([P, 32], i16)
        # SWDGE (gpsimd) so these never stall the sync sequencer's load
        # stream; a DRAM round-trip beats SBUF hop-trees: DRAM APs can
        # broadcast, so the whole distribute is 2 triggers instead of 23
        if IDX_DRAM:
            # scalar (ACT) HWDGE ring: lower trigger latency than SWDGE
            # hops and keeps gpsimd free for the scatter
            nc.scalar.dma_start(out=scr_d, in_=idxs_i[0:1, :])
            nc.scalar.dma_start(
                out=idxs_all[:],
                in_=scr_d.rearrange("a (c i) -> (a c) i", c=16)[None, :, :]
                    .to_broadcast((8, 16, 32)))
        else:
            nc.gpsimd.dma_start(out=idxs_all[0:16, :], in_=idxs_i[0:1, :])
            nc.gpsimd.dma_start(out=idxs_all[16:32, :], in_=idxs_all[0:16, :])
            nc.gpsimd.dma_start(out=idxs_all[32:64, :], in_=idxs_all[0:32, :])
            nc.gpsimd.dma_start(out=idxs_all[64:128, :], in_=idxs_all[0:64, :])

        # ---- scatter adds: pair-packed, both lanes per entry, sigma-permuted
        # (reuses the "pair" buffer: ids_bc_i is dead once pair_f exists)
        add_pairs = work.tile([P, S, 2], bf16, tag="pair", bufs=1)
        add_v = add_pairs[:].rearrange("c (i p) d -> c p i d", p=16)
        nc.vector.tensor_scalar(add_v[:, :, :, 0],
                                comb_e[:].rearrange("c (p i) -> c p i", p=16),
                                s1[:], None, AluOp.mult)
        nc.vector.tensor_scalar(add_v[:, :, :, 1],
                                comb_o[:].rearrange("c (p i) -> c p i", p=16),
                                s1[:], None, AluOp.mult)
        nc.gpsimd.scatter_add(in_ap=pcopy[:], idxs_ap=idxs_all[:],
                              add_ap=add_pairs[:], channels=P, num_elems=NPAIR,
                              d=2, num_idxs=S)

        # ---- pass 1 tail: remaining exps (loads already in flight) ----
        for k in range(NEARLY, NT):
            off = k * VTILE
            w_k = min(VTILE, V - off)
            nc.scalar.activation(out=exp_store[:, off:off + w_k],
                                 in_=ltiles[k][:, :w_k], func=Act.Exp)
            nc.vector.tensor_reduce(out=zparts[:, k:k + 1],
                                    in_=exp_store[:, off:off + w_k],
                                    axis=mybir.AxisListType.X, op=AluOp.add)

        # ---- softmax scale ----
        Z = fix.tile([P, 1], f32)
        nc.vector.tensor_reduce(out=Z[:], in_=zparts[:], axis=mybir.AxisListType.X,
                                op=AluOp.add)
        invZ = fix.tile([P, 1], f32)
        nc.vector.reciprocal(out=invZ[:], in_=Z[:])
        s0 = fix.tile([P, 1], f32)
        nc.vector.tensor_mul(out=s0[:], in0=p_gen[:], in1=invZ[:])

        # ---- pass 2: fused all-bf16 blend IN PLACE over exp_store, then
        # cast-on-store straight from it. Regions are disjoint, so no
        # rotation buffers and the SWDGE queue can hold all 32 transfers.
        pcopy_flat = pcopy[:].rearrange("p a b -> p (a b)")
        # tail (smallest) tile first so the final store transfer is full-size
        for k in [NT - 1] + list(range(NT - 1)):
            off = k * VTILE
            w_k = min(VTILE, V - off)
            nc.vector.scalar_tensor_tensor(
                out=exp_store[:, off:off + w_k],
                in0=exp_store[:, off:off + w_k], scalar=s0[:],
                in1=pcopy_flat[:, off:off + w_k], op0=AluOp.mult, op1=AluOp.add)
            # SWDGE casts bf16 -> f32 on the way out
            nc.gpsimd.dma_start(out=out_d[:, off:off + w_k],
                                in_=exp_store[:, off:off + w_k])


_CACHE = {}


def _get_graph():
    if "nc" in _CACHE:
        return _CACHE["nc"]
    nc = bacc.Bacc("TRN2", target_bir_lowering=False, debug=False,
                   num_devices=NCORES)
    ids_d = nc.dram_tensor("ids", [S], i32, kind="ExternalInput").ap()
    logits_d = nc.dram_tensor("logits", [P, V], f32, kind="ExternalInput").ap()
    enc_d = nc.dram_tensor("enc", [S, D], f32, kind="ExternalInput").ap()
    dec_d = nc.dram_tensor("dec", [P, D], f32, kind="ExternalInput").ap()
    xattn_d = nc.dram_tensor("xattn", [H, P, S], f32, kind="ExternalInput").ap()
    wgw_d = nc.dram_tensor("wgw", [1, 2 * D], f32, kind="ExternalInput").ap()
    wgb_d = nc.dram_tensor("wgb", [1], f32, kind="ExternalInput").ap()
    scr_d = nc.dram_tensor("idx_scratch", [1, S], i16, kind="ExternalOutput").ap()
    out_d = nc.dram_tensor("out", [P, V], f32, kind="ExternalOutput").ap()
    with TileContext(nc) as tc:
        _body(tc, ids_d, logits_d, enc_d, dec_d, xattn_d, wgw_d, wgb_d, scr_d,
              out_d)
    nc.compile()
    _CACHE["nc"] = nc
    return nc


def _shard(inputs):
    ids = np.asarray(inputs["input_ids"])
    logits = np.asarray(inputs["logits"], dtype=np.float32)
    enc = np.asarray(inputs["encoder_hidden_states"], dtype=np.float32)
    dec = np.asarray(inputs["decoder_hidden_states"], dtype=np.float32)
    xattn = np.asarray(inputs["cross_attentions"], dtype=np.float32)
    wgw = np.asarray(inputs["W_gen_w"], dtype=np.float32)
    wgb = np.asarray(inputs["W_gen_b"], dtype=np.float32)
    in_maps = []
    for c in range(NCORES):
        b, th = c // 2, c % 2
        t0 = th * P
        in_maps.append({
            "ids": np.ascontiguousarray(ids[b]).astype(np.int32),
            "logits": np.ascontiguousarray(logits[b, t0:t0 + P, :]),
            "enc": np.ascontiguousarray(enc[b]),
            "dec": np.ascontiguousarray(dec[b, t0:t0 + P, :]),
            "xattn": np.ascontiguousarray(xattn[b, :, t0:t0 + P, :]),
            "wgw": wgw,
            "wgb": wgb,
        })
    return in_maps


def run(inputs, trace=False):
    nc = _get_graph()
    in_maps = _shard(inputs)
    res = bass_utils.run_bass_kernel_spmd(nc, in_maps,
                                          core_ids=list(range(NCORES)),
                                          trace=trace)
    out = np.empty((B, T, V), np.float32)
    for c in range(NCORES):
        b, th = c // 2, c % 2
        out[b, th * P:(th + 1) * P, :] = res.results[c]["out"]  # [P, V]
    return out, res


def kernel(**inputs):
    out, _ = run(inputs, trace=False)
    return out



# revision 59
# speedup vs baseline: 1.2598x; 1.0534x over previous
"""Copy-enhanced CodeT5 head (histogram/scatter blend) on 8 TRN2 NeuronCores.

Strategy: data-parallel over (batch, T/2) -> 8 shards of 128 decoder rows.
Each core, for its [128, V] output block:
  A_sum    = sum_h cross_attn[h]                       (DVE adds)
  p_gen    = sigmoid((A_sum @ (enc @ W1))/H + dec.W2 + b)   (PE + DVE dots + ACT)
  exp, Z   = exp(logits) streamed, row-sums via ACT accum   (pass 1)
  P_copy   = scatter-add of (1-p_gen)/H * (A_sum @ Sel) into a bf16
             pair-packed accumulator via gpsimd scatter_add; duplicate
             source ids are pre-combined with a selection-matrix matmul
             and non-first occurrences are redirected to a dump slot
             (the hardware scatter pipeline does not accumulate racing
             duplicate indices).
  out      = exp * (p_gen/Z) + P_copy                  (one fused DVE op, pass 2)

No collectives needed: every core owns a disjoint output block.
"""
import sys

sys.path.insert(0, "/opt/trn_rl_repo")

import numpy as np

import concourse.bass as bass  # noqa: F401  (registers engine classes)
import concourse.mybir as mybir
from concourse import bacc, bass_utils, library_config
from concourse.tile import TileContext
from concourse.masks import make_identity

B, S, T, D, H, V = 4, 512, 256, 1024, 16, 32105
P = 128
NCORES = 8
NPAIR = V // 2 + 2          # 16054 pair slots; pairs 0..16052 hold vocab, 16053 = dump
DUMP = NPAIR - 1
VTILE = 1024
NT = (V + VTILE - 1) // VTILE
NEARLY = 8
LIB_PRELOAD = True
IDX_DRAM = True

AluOp = mybir.AluOpType
Act = mybir.ActivationFunctionType
f32 = mybir.dt.float32
bf16 = mybir.dt.bfloat16
i32 = mybir.dt.int32
i16 = mybir.dt.int16


def _body(tc, ids_d, logits_d, enc_d, dec_d, xattn_d, wgw_d, wgb_d, scr_d,
          out_d):
    nc = tc.nc
    with tc.tile_pool(name="fix", bufs=1) as fix, \
         tc.tile_pool(name="work", bufs=4) as work, \
         tc.tile_pool(name="lpool", bufs=4) as lpool, \
         tc.tile_pool(name="psum", bufs=1, space="PSUM") as psum:

        A = fix.tile([P, S], f32)

        # preload the gpsimd ucode library that scatter_add needs; the ~13us
        # load runs under the input-DMA phase instead of gating the scatter
        if LIB_PRELOAD:
            nc.gpsimd.load_library(library_config.mlp)

        # ---- persistent tiles ----
        exp_store = fix.tile([P, V], bf16)
        pcopy = fix.tile([P, NPAIR, 2], bf16)
        # zero the accumulator on ACT (otherwise idle before the exps);
        # emitted first so the DVE prologue chain stays unblocked
        nc.scalar.memzero(pcopy[:])

        ident = fix.tile([P, P], f32)
        make_identity(nc, ident[:])

        # ---- ALL input DMAs up front, COALESCED (the sync sequencer takes
        # ~0.6us per trigger: 25 triggers used to delay the ltile stream by
        # 21us). ids first so the pair/Sel chain starts immediately. No
        # trigger here may block on a consumer, or it head-of-line-stalls
        # the sync sequencer and starves everything behind it.
        ids_bc_i = work.tile([P, S], i32, tag="pair", bufs=1)
        nc.sync.dma_start(out=ids_bc_i[:], in_=ids_d[None, :].to_broadcast((P, S)))
        ids_col_i = fix.tile([P, 4], i32)
        nc.sync.dma_start(out=ids_col_i[:], in_=ids_d.rearrange("(c p) -> p c", p=P))
        wg = work.tile([P, 2 * D], f32, tag="wgt", bufs=1)
        nc.sync.dma_start(out=wg[:], in_=wgw_d[0:1, :].to_broadcast((P, 2 * D)))
        wb_bc = fix.tile([P, 1], f32)
        nc.sync.dma_start(out=wb_bc[:], in_=wgb_d[None, :].to_broadcast((P, 1)))
        dec_t = work.tile([P, D], f32, tag="dec", bufs=1)
        nc.sync.dma_start(out=dec_t[:], in_=dec_d[:])
        enc_g = work.tile([P, 4, D], f32, tag="enc", bufs=1)
        nc.sync.dma_start(out=enc_g[:], in_=enc_d.rearrange("(c p) d -> p c d", p=P))
        # head groups: hg0/hg1 triggered from sync; hg2/hg3 from the scalar
        # ring (they block on hg0/hg1 consumption, and a blocked trigger
        # must not sit ahead of the ltile triggers in the sync stream)
        hgs = []
        for g in range(4):
            hg = work.tile([P, 4, S], f32, tag="wk", name=f"hg{g}", bufs=2)
            eng = nc.sync if g < 2 else nc.scalar
            eng.dma_start(out=hg[:],
                          in_=xattn_d[4 * g:4 * g + 4].rearrange("h p s -> p h s"))
            hgs.append(hg)

        # ---- pass-1 load stream issued NOW (before any compute-dependent
        # DMAs can stall the sync sequencer); first few exps too, so the
        # p_gen exp below lands between exp11 and exp12 on the in-order ACT
        zparts = fix.tile([P, NT], f32)
        ltiles = []
        for k in range(NT):
            ltile = lpool.tile([P, VTILE], f32, tag="lt", name=f"lt{k}")
            w_k = min(VTILE, V - k * VTILE)
            nc.sync.dma_start(out=ltile[:, :w_k],
                              in_=logits_d[:, k * VTILE:k * VTILE + w_k])
            ltiles.append(ltile)
        # exps WITHOUT the ACT accumulator read (which costs 279ns/tile and
        # made ACT the load-stream pacer); row-sums go to DVE instead
        for k in range(NEARLY):
            off = k * VTILE
            w_k = min(VTILE, V - off)
            nc.scalar.activation(out=exp_store[:, off:off + w_k],
                                 in_=ltiles[k][:, :w_k], func=Act.Exp)
            nc.vector.tensor_reduce(out=zparts[:, k:k + 1],
                                    in_=exp_store[:, off:off + w_k],
                                    axis=mybir.AxisListType.X, op=AluOp.add)

        # head sum -> A (serial: DVE keeps pace with head arrival)
        nc.vector.tensor_copy(out=A[:], in_=hgs[0][:, 0, :])
        for h in range(1, H):
            nc.vector.tensor_add(out=A[:], in0=A[:],
                                 in1=hgs[h // 4][:, h % 4, :])

        # ---- A^T via PE transposes (bf16: feeds bf16 matmuls/scatter) ----
        A_T = fix.tile([P, 4, P], bf16)
        for kk in range(4):
            tps = psum.tile([P, P], f32, tag="tps", bufs=2, name=f"tps{kk}")
            nc.tensor.transpose(tps[:], A[:, kk * P:(kk + 1) * P], ident[:])
            nc.vector.tensor_copy(out=A_T[:, kk, :], in_=tps[:])

        # ---- p_gen (emit early: its sigmoid must precede the exps on ACT) ----
        u_col = fix.tile([P, 4], f32)
        for kk in range(4):
            junk = work.tile([P, D], bf16, tag="jnk", name=f"junk{kk}", bufs=1)
            nc.vector.scalar_tensor_tensor(out=junk[:], in0=enc_g[:, kk, :],
                                           scalar=1.0,
                                           in1=wg[:, 0:D], op0=AluOp.mult,
                                           op1=AluOp.mult,
                                           accum_out=u_col[:, kk:kk + 1])
        u_colb = fix.tile([P, 4], bf16)
        nc.vector.tensor_copy(out=u_colb[:], in_=u_col[:])
        plin1_ps = psum.tile([P, 1], f32, tag="plin")
        for kk in range(4):
            nc.tensor.matmul(plin1_ps[:], A_T[:, kk, :], u_colb[:, kk:kk + 1],
                             start=(kk == 0), stop=(kk == 3))
        p_lin2 = fix.tile([P, 1], f32)
        junk2 = work.tile([P, D], bf16, tag="jnk", bufs=1)
        nc.vector.scalar_tensor_tensor(out=junk2[:], in0=dec_t[:], scalar=1.0,
                                       in1=wg[:, D:2 * D], op0=AluOp.mult,
                                       op1=AluOp.mult, accum_out=p_lin2[:])
        p_lin2b = fix.tile([P, 1], f32)
        nc.vector.tensor_add(out=p_lin2b[:], in0=p_lin2[:], in1=wb_bc[:])
        # sigmoid via the already-loaded exp table (avoids two mid-stream
        # ACT table swaps): p_gen = e/(1+e), e = exp(plin1/H + p_lin2b)
        e_sig = fix.tile([P, 1], f32)
        nc.scalar.activation(out=e_sig[:], in_=plin1_ps[:], func=Act.Exp,
                             bias=p_lin2b[:], scale=1.0 / H)
        e1 = fix.tile([P, 1], f32)
        nc.vector.tensor_scalar(e1[:], e_sig[:], 1.0, None, AluOp.add)
        e1r = fix.tile([P, 1], f32)
        nc.vector.reciprocal(out=e1r[:], in_=e1[:])
        p_gen = fix.tile([P, 1], f32)
        nc.vector.tensor_mul(out=p_gen[:], in0=e_sig[:], in1=e1r[:])
        s1 = fix.tile([P, 1], f32)
        nc.vector.tensor_scalar(s1[:], p_gen[:], -1.0 / H, 1.0 / H,
                                AluOp.mult, AluOp.add)

        # ---- pair-level selection matrix + per-lane combine (int compares,
        # bf16 masks: halves SBUF vs the f32 originals) ----
        # shift the ids broadcast to pair ids in place, then convert to f32
        nc.vector.tensor_scalar(ids_bc_i[:], ids_bc_i[:], 1, None,
                                AluOp.arith_shift_right)
        pair_f = work.tile([P, S], f32, tag="pair2", bufs=1)
        nc.vector.tensor_copy(out=pair_f[:], in_=ids_bc_i[:])
        parity_ci = fix.tile([P, 4], i32)
        nc.vector.tensor_scalar(parity_ci[:], ids_col_i[:], 1, None,
                                AluOp.bitwise_and)
        parity_colf = fix.tile([P, 4], f32)
        nc.vector.tensor_copy(out=parity_colf[:], in_=parity_ci[:])
        nc.vector.tensor_scalar(ids_col_i[:], ids_col_i[:], 1, None,
                                AluOp.arith_shift_right)
        pair_colf = fix.tile([P, 4], f32)
        nc.vector.tensor_copy(out=pair_colf[:], in_=ids_col_i[:])
        par_is = fix.tile([P, 4, 2], f32)
        nc.vector.tensor_scalar(par_is[:, :, 0], parity_colf[:], 0.0, None,
                                AluOp.is_equal)
        nc.vector.tensor_scalar(par_is[:, :, 1], parity_colf[:], 1.0, None,
                                AluOp.is_equal)
        Sel = fix.tile([P, 4, S], bf16)
        for kk in range(4):
            nc.vector.tensor_scalar(Sel[:, kk, :], pair_f[:],
                                    pair_colf[:, kk:kk + 1], None, AluOp.is_equal)
        m2 = fix.tile([P, S], bf16)
        comb_e = psum.tile([P, S], f32, tag="combe")
        comb_o = psum.tile([P, S], f32, tag="combo")
        for lane, comb_ps_l in ((0, comb_e), (1, comb_o)):
            for kk in range(4):
                nc.vector.tensor_scalar(m2[:], Sel[:, kk, :],
                                        par_is[:, kk:kk + 1, lane], None, AluOp.mult)
                nc.tensor.matmul(comb_ps_l[:], A_T[:, kk, :], m2[:],
                                 start=(kk == 0), stop=(kk == 3))
        # lower-triangular mask (strictly s' < s), in place; Sel becomes LSel
        for kk in range(4):
            nc.gpsimd.affine_select(
                out=Sel[:, kk, :], in_=Sel[:, kk, :],
                pattern=[[1, S]], compare_op=AluOp.is_ge, fill=0.0,
                base=-(kk * P) - 1, channel_multiplier=-1,
            )
        ones_t = fix.tile([P, 1], bf16)
        nc.vector.memset(ones_t[:], 1.0)
        dup_ps = psum.tile([1, S], f32, tag="dup")
        for kk in range(4):
            nc.tensor.matmul(dup_ps[:], ones_t[:], Sel[:, kk, :],
                             start=(kk == 0), stop=(kk == 3))
        first_occ = fix.tile([1, S], f32)
        nc.vector.tensor_scalar(first_occ[:], dup_ps[:], 0.0, None, AluOp.is_equal)

        # ---- scatter index row: first pair-occurrence -> pair slot, else dump ----
        d1 = fix.tile([1, S], f32)
        nc.vector.tensor_scalar(d1[:], pair_f[0:1, :], -float(DUMP), None,
                                AluOp.add)
        nc.vector.scalar_tensor_tensor(out=d1[:], in0=d1[:], scalar=1.0,
                                       in1=first_occ[:], op0=AluOp.mult,
                                       op1=AluOp.mult)
        nc.vector.tensor_scalar(d1[:], d1[:], float(DUMP), None, AluOp.add)
        idxs_i = fix.tile([1, S], i16)
        nc.vector.tensor_copy(out=idxs_i[:], in_=d1[:])
        # distribute [1, 512] -> [128, 32] in CHUNKED layout: tile[p, i] =
        # row[p*32 + i]; list position j maps to source column
        # sigma(j) = (j % 16)*32 + j // 16 (adds written sigma-permuted below)
        idxs_all = fix.tile([P, 32], i16)
        # SWDGE (gpsimd) so these never stall the sync sequencer's load
        # stream; a DRAM round-trip beats SBUF hop-trees: DRAM APs can
        # broadcast, so the whole distribute is 2 triggers instead of 23
        if IDX_DRAM:
            # scalar (ACT) HWDGE ring: lower trigger latency than SWDGE
            # hops and keeps gpsimd free for the scatter
            nc.scalar.dma_start(out=scr_d, in_=idxs_i[0:1, :])
            nc.scalar.dma_start(
                out=idxs_all[:],
                in_=scr_d.rearrange("a (c i) -> (a c) i", c=16)[None, :, :]
                    .to_broadcast((8, 16, 32)))
        else:
            nc.gpsimd.dma_start(out=idxs_all[0:16, :], in_=idxs_i[0:1, :])
            nc.gpsimd.dma_start(out=idxs_all[16:32, :], in_=idxs_all[0:16, :])
            nc.gpsimd.dma_start(out=idxs_all[32:64, :], in_=idxs_all[0:32, :])
            nc.gpsimd.dma_start(out=idxs_all[64:128, :], in_=idxs_all[0:64, :])

        # ---- scatter adds: pair-packed, both lanes per entry, sigma-permuted
        # (reuses the "pair" buffer: ids_bc_i is dead once pair_f exists)
        add_pairs = work.tile([P, S, 2], bf16, tag="pair", bufs=1)
        add_v = add_pairs[:].rearrange("c (i p) d -> c p i d", p=16)
        nc.vector.tensor_scalar(add_v[:, :, :, 0],
                                comb_e[:].rearrange("c (p i) -> c p i", p=16),
                                s1[:], None, AluOp.mult)
        nc.vector.tensor_scalar(add_v[:, :, :, 1],
                                comb_o[:].rearrange("c (p i) -> c p i", p=16),
                                s1[:], None, AluOp.mult)
        nc.gpsimd.scatter_add(in_ap=pcopy[:], idxs_ap=idxs_all[:],
                              add_ap=add_pairs[:], channels=P, num_elems=NPAIR,
                              d=2, num_idxs=S)

        # ---- pass 1 tail: remaining exps (loads already in flight) ----
        for k in range(NEARLY, NT):
            off = k * VTILE
            w_k = min(VTILE, V - off)
            nc.scalar.activation(out=exp_store[:, off:off + w_k],
                                 in_=ltiles[k][:, :w_k], func=Act.Exp)
            nc.vector.tensor_reduce(out=zparts[:, k:k + 1],
                                    in_=exp_store[:, off:off + w_k],
                                    axis=mybir.AxisListType.X, op=AluOp.add)

        # ---- softmax scale ----
        Z = fix.tile([P, 1], f32)
        nc.vector.tensor_reduce(out=Z[:], in_=zparts[:], axis=mybir.AxisListType.X,
                                op=AluOp.add)
        invZ = fix.tile([P, 1], f32)
        nc.vector.reciprocal(out=invZ[:], in_=Z[:])
        s0 = fix.tile([P, 1], f32)
        nc.vector.tensor_mul(out=s0[:], in0=p_gen[:], in1=invZ[:])

        # ---- pass 2: fused all-bf16 blend IN PLACE over exp_store, then
        # cast-on-store straight from it. Regions are disjoint, so no
        # rotation buffers and the SWDGE queue can hold all 32 transfers.
        pcopy_flat = pcopy[:].rearrange("p a b -> p (a b)")
        # tail (smallest) tile first so the final store transfer is full-size
        for k in [NT - 1] + list(range(NT - 1)):
            off = k * VTILE
            w_k = min(VTILE, V - off)
            nc.vector.scalar_tensor_tensor(
                out=exp_store[:, off:off + w_k],
                in0=exp_store[:, off:off + w_k], scalar=s0[:],
                in1=pcopy_flat[:, off:off + w_k], op0=AluOp.mult, op1=AluOp.add)
            # SWDGE casts bf16 -> f32 on the way out
            nc.gpsimd.dma_start(out=out_d[:, off:off + w_k],
                                in_=exp_store[:, off:off + w_k])


_CACHE = {}


def _get_graph():
    if "nc" in _CACHE:
        return _CACHE["nc"]
    nc = bacc.Bacc("TRN2", target_bir_lowering=False, debug=False,
                   num_devices=NCORES)
    ids_d = nc.dram_tensor("ids", [S], i32, kind="ExternalInput").ap()
    logits_d = nc.dram_tensor("logits", [P, V], f32, kind="ExternalInput").ap()
    enc_d = nc.dram_tensor("enc", [S, D], f32, kind="ExternalInput").ap()
    dec_d = nc.dram_tensor("dec", [P, D], f32, kind="ExternalInput").ap()
    xattn_d = nc.dram_tensor("xattn", [H, P, S], f32, kind="ExternalInput").ap()
    wgw_d = nc.dram_tensor("wgw", [1, 2 * D], f32, kind="ExternalInput").ap()
    wgb_d = nc.dram_tensor("wgb", [1], f32, kind="ExternalInput").ap()
    scr_d = nc.dram_tensor("idx_scratch", [1, S], i16, kind="ExternalOutput").ap()
    out_d = nc.dram_tensor("out", [P, V], f32, kind="ExternalOutput").ap()
    with TileContext(nc) as tc:
        _body(tc, ids_d, logits_d, enc_d, dec_d, xattn_d, wgw_d, wgb_d, scr_d,
              out_d)
    nc.compile()
    _CACHE["nc"] = nc
    return nc


def _shard(inputs):
    ids = np.asarray(inputs["input_ids"])
    logits = np.asarray(inputs["logits"], dtype=np.float32)
    enc = np.asarray(inputs["encoder_hidden_states"], dtype=np.float32)
    dec = np.asarray(inputs["decoder_hidden_states"], dtype=np.float32)
    xattn = np.asarray(inputs["cross_attentions"], dtype=np.float32)
    wgw = np.asarray(inputs["W_gen_w"], dtype=np.float32)
    wgb = np.asarray(inputs["W_gen_b"], dtype=np.float32)
    in_maps = []
    for c in range(NCORES):
        b, th = c // 2, c % 2
        t0 = th * P
        in_maps.append({
            "ids": np.ascontiguousarray(ids[b]).astype(np.int32),
            "logits": np.ascontiguousarray(logits[b, t0:t0 + P, :]),
            "enc": np.ascontiguousarray(enc[b]),
            "dec": np.ascontiguousarray(dec[b, t0:t0 + P, :]),
            "xattn": np.ascontiguousarray(xattn[b, :, t0:t0 + P, :]),
            "wgw": wgw,
            "wgb": wgb,
        })
    return in_maps


def run(inputs, trace=False):
    nc = _get_graph()
    in_maps = _shard(inputs)
    res = bass_utils.run_bass_kernel_spmd(nc, in_maps,
                                          core_ids=list(range(NCORES)),
                                          trace=trace)
    out = np.empty((B, T, V), np.float32)
    for c in range(NCORES):
        b, th = c // 2, c % 2
        out[b, th * P:(th + 1) * P, :] = res.results[c]["out"]  # [P, V]
    return out, res


def kernel(**inputs):
    out, _ = run(inputs, trace=False)
    return out



# revision 64
# speedup vs baseline: 1.3366x; 1.0610x over previous
"""Copy-enhanced CodeT5 head (histogram/scatter blend) on 8 TRN2 NeuronCores.

Strategy: data-parallel over (batch, T/2) -> 8 shards of 128 decoder rows.
Each core, for its [128, V] output block:
  A_sum    = sum_h cross_attn[h]                       (DVE adds)
  p_gen    = sigmoid((A_sum @ (enc @ W1))/H + dec.W2 + b)   (PE + DVE dots + ACT)
  exp, Z   = exp(logits) streamed, row-sums via ACT accum   (pass 1)
  P_copy   = scatter-add of (1-p_gen)/H * (A_sum @ Sel) into a bf16
             pair-packed accumulator via gpsimd scatter_add; duplicate
             source ids are pre-combined with a selection-matrix matmul
             and non-first occurrences are redirected to a dump slot
             (the hardware scatter pipeline does not accumulate racing
             duplicate indices).
  out      = exp * (p_gen/Z) + P_copy                  (one fused DVE op, pass 2)

No collectives needed: every core owns a disjoint output block.
"""
import sys

sys.path.insert(0, "/opt/trn_rl_repo")

import numpy as np

import concourse.bass as bass  # noqa: F401  (registers engine classes)
import concourse.mybir as mybir
from concourse import bacc, bass_utils, library_config
from concourse.tile import TileContext
from concourse.masks import make_identity

B, S, T, D, H, V = 4, 512, 256, 1024, 16, 32105
P = 128
NCORES = 8
NPAIR = V // 2 + 2          # 16054 pair slots; pairs 0..16052 hold vocab, 16053 = dump
DUMP = NPAIR - 1
VTILE = 1024
NT = (V + VTILE - 1) // VTILE
NEARLY = 10
LIB_PRELOAD = True
IDX_DRAM = True

AluOp = mybir.AluOpType
Act = mybir.ActivationFunctionType
f32 = mybir.dt.float32
bf16 = mybir.dt.bfloat16
i32 = mybir.dt.int32
i16 = mybir.dt.int16


def _body(tc, ids_d, logits_d, enc_d, dec_d, xattn_d, wgw_d, wgb_d, scr_d,
          out_d):
    nc = tc.nc
    with tc.tile_pool(name="fix", bufs=1) as fix, \
         tc.tile_pool(name="work", bufs=4) as work, \
         tc.tile_pool(name="psum", bufs=1, space="PSUM") as psum:

        # ---- persistent tiles ----
        exp_store = fix.tile([P, V], bf16)
        pcopy = fix.tile([P, NPAIR, 2], bf16)
        A = fix.tile([P, S], f32)

        # ---- logits stream: SWDGE CAST-ON-LOAD straight into disjoint
        # slices of the bf16 exp_store. No rotation buffers -> no
        # consumer round-trips -> the Q0 queues stay saturated like the
        # store phase does (~400 GB/s), concurrent with Q1's misc loads.
        # The exp then runs IN PLACE over each slice. First half of the
        # triggers now; second half after the gpsimd library load below.
        for k in range(NT // 2):
            off = k * VTILE
            w_k = min(VTILE, V - off)
            nc.gpsimd.dma_start(out=exp_store[:, off:off + w_k],
                                in_=logits_d[:, off:off + w_k])

        # preload the gpsimd ucode library that scatter_add needs (~15us,
        # blocks gpsimd): slotted here so the first trigger batch already
        # queued ~30us of DMA work
        if LIB_PRELOAD:
            nc.gpsimd.load_library(library_config.mlp)

        # zero the scatter accumulator: half on ACT, half on DVE, so the
        # exps can start ~7us earlier than a full-width ACT memzero allows
        nc.scalar.memzero(pcopy[:, :NPAIR // 2, :])
        nc.vector.memset(pcopy[:, NPAIR // 2:, :], 0)

        ident = fix.tile([P, P], f32)
        make_identity(nc, ident[:])

        # ---- Q1 input DMAs, COALESCED (the sync sequencer takes ~0.6us
        # per trigger). ids first so the pair/Sel chain starts right away,
        # then heads (the A -> scatter critical chain), then the rest.
        ids_bc_i = work.tile([P, S], i32, tag="pair", bufs=1)
        nc.sync.dma_start(out=ids_bc_i[:], in_=ids_d[None, :].to_broadcast((P, S)))
        ids_col_i = fix.tile([P, 4], i32)
        nc.sync.dma_start(out=ids_col_i[:], in_=ids_d.rearrange("(c p) -> p c", p=P))
        hgs = []
        for g in range(4):
            hg = work.tile([P, 4, S], f32, tag="wk", name=f"hg{g}", bufs=3)
            nc.sync.dma_start(out=hg[:],
                              in_=xattn_d[4 * g:4 * g + 4].rearrange("h p s -> p h s"))
            hgs.append(hg)
        wg = work.tile([P, 2 * D], f32, tag="wgt", bufs=1)
        nc.sync.dma_start(out=wg[:], in_=wgw_d[0:1, :].to_broadcast((P, 2 * D)))
        wb_bc = fix.tile([P, 1], f32)
        nc.sync.dma_start(out=wb_bc[:], in_=wgb_d[None, :].to_broadcast((P, 1)))
        dec_t = work.tile([P, D], f32, tag="dec", bufs=1)
        nc.sync.dma_start(out=dec_t[:], in_=dec_d[:])
        enc_g = work.tile([P, 4, D], f32, tag="enc", bufs=1)
        nc.sync.dma_start(out=enc_g[:], in_=enc_d.rearrange("(c p) d -> p c d", p=P))

        # ---- exps in place, row-sums via the ACT accumulator (ACT has
        # slack now: nothing downstream paces on it)
        zparts = fix.tile([P, NT], f32)
        for k in range(NEARLY):
            off = k * VTILE
            w_k = min(VTILE, V - off)
            nc.scalar.activation(out=exp_store[:, off:off + w_k],
                                 in_=exp_store[:, off:off + w_k],
                                 func=Act.Exp, accum_out=zparts[:, k:k + 1])

        # head sum -> A (serial: DVE keeps pace with head arrival)
        nc.vector.tensor_copy(out=A[:], in_=hgs[0][:, 0, :])
        for h in range(1, H):
            nc.vector.tensor_add(out=A[:], in0=A[:],
                                 in1=hgs[h // 4][:, h % 4, :])

        # ---- A^T via PE transposes (bf16: feeds bf16 matmuls/scatter) ----
        A_T = fix.tile([P, 4, P], bf16)
        for kk in range(4):
            tps = psum.tile([P, P], f32, tag="tps", bufs=2, name=f"tps{kk}")
            nc.tensor.transpose(tps[:], A[:, kk * P:(kk + 1) * P], ident[:])
            nc.vector.tensor_copy(out=A_T[:, kk, :], in_=tps[:])

        # ---- p_gen (emit early: its sigmoid must precede the exps on ACT) ----
        u_col = fix.tile([P, 4], f32)
        for kk in range(4):
            junk = work.tile([P, D], bf16, tag="jnk", name=f"junk{kk}", bufs=1)
            nc.vector.scalar_tensor_tensor(out=junk[:], in0=enc_g[:, kk, :],
                                           scalar=1.0,
                                           in1=wg[:, 0:D], op0=AluOp.mult,
                                           op1=AluOp.mult,
                                           accum_out=u_col[:, kk:kk + 1])
        u_colb = fix.tile([P, 4], bf16)
        nc.vector.tensor_copy(out=u_colb[:], in_=u_col[:])
        plin1_ps = psum.tile([P, 1], f32, tag="plin")
        for kk in range(4):
            nc.tensor.matmul(plin1_ps[:], A_T[:, kk, :], u_colb[:, kk:kk + 1],
                             start=(kk == 0), stop=(kk == 3))
        p_lin2 = fix.tile([P, 1], f32)
        junk2 = work.tile([P, D], bf16, tag="jnk", bufs=1)
        nc.vector.scalar_tensor_tensor(out=junk2[:], in0=dec_t[:], scalar=1.0,
                                       in1=wg[:, D:2 * D], op0=AluOp.mult,
                                       op1=AluOp.mult, accum_out=p_lin2[:])
        p_lin2b = fix.tile([P, 1], f32)
        nc.vector.tensor_add(out=p_lin2b[:], in0=p_lin2[:], in1=wb_bc[:])
        # sigmoid via the already-loaded exp table (avoids two mid-stream
        # ACT table swaps): p_gen = e/(1+e), e = exp(plin1/H + p_lin2b)
        e_sig = fix.tile([P, 1], f32)
        nc.scalar.activation(out=e_sig[:], in_=plin1_ps[:], func=Act.Exp,
                             bias=p_lin2b[:], scale=1.0 / H)
        e1 = fix.tile([P, 1], f32)
        nc.vector.tensor_scalar(e1[:], e_sig[:], 1.0, None, AluOp.add)
        e1r = fix.tile([P, 1], f32)
        nc.vector.reciprocal(out=e1r[:], in_=e1[:])
        p_gen = fix.tile([P, 1], f32)
        nc.vector.tensor_mul(out=p_gen[:], in0=e_sig[:], in1=e1r[:])
        s1 = fix.tile([P, 1], f32)
        nc.vector.tensor_scalar(s1[:], p_gen[:], -1.0 / H, 1.0 / H,
                                AluOp.mult, AluOp.add)

        # ---- pair-level selection matrix + per-lane combine (int compares,
        # bf16 masks: halves SBUF vs the f32 originals) ----
        # shift the ids broadcast to pair ids in place, then convert to f32
        nc.vector.tensor_scalar(ids_bc_i[:], ids_bc_i[:], 1, None,
                                AluOp.arith_shift_right)
        pair_f = work.tile([P, S], f32, tag="pair2", bufs=1)
        nc.vector.tensor_copy(out=pair_f[:], in_=ids_bc_i[:])
        parity_ci = fix.tile([P, 4], i32)
        nc.vector.tensor_scalar(parity_ci[:], ids_col_i[:], 1, None,
                                AluOp.bitwise_and)
        parity_colf = fix.tile([P, 4], f32)
        nc.vector.tensor_copy(out=parity_colf[:], in_=parity_ci[:])
        nc.vector.tensor_scalar(ids_col_i[:], ids_col_i[:], 1, None,
                                AluOp.arith_shift_right)
        pair_colf = fix.tile([P, 4], f32)
        nc.vector.tensor_copy(out=pair_colf[:], in_=ids_col_i[:])
        par_is = fix.tile([P, 4, 2], f32)
        nc.vector.tensor_scalar(par_is[:, :, 0], parity_colf[:], 0.0, None,
                                AluOp.is_equal)
        nc.vector.tensor_scalar(par_is[:, :, 1], parity_colf[:], 1.0, None,
                                AluOp.is_equal)
        Sel = fix.tile([P, 4, S], bf16)
        for kk in range(4):
            nc.vector.tensor_scalar(Sel[:, kk, :], pair_f[:],
                                    pair_colf[:, kk:kk + 1], None, AluOp.is_equal)

        # lower-triangular-masked copy (strictly s' < s) into a SEPARATE
        # tile so it can run as soon as Sel exists (no WAR on the comb
        # matmuls), early on gpsimd right after the library load
        LSel = fix.tile([P, 4, S], bf16)
        for kk in range(4):
            nc.gpsimd.affine_select(
                out=LSel[:, kk, :], in_=Sel[:, kk, :],
                pattern=[[1, S]], compare_op=AluOp.is_ge, fill=0.0,
                base=-(kk * P) - 1, channel_multiplier=-1,
            )
        # second batch of cast-on-load triggers (gpsimd was busy with the
        # library load; the first batch kept the Q0 queues fed meanwhile)
        for k in range(NT // 2, NT):
            off = k * VTILE
            w_k = min(VTILE, V - off)
            nc.gpsimd.dma_start(out=exp_store[:, off:off + w_k],
                                in_=logits_d[:, off:off + w_k])

        m2 = fix.tile([P, S], bf16)
        comb_e = psum.tile([P, S], f32, tag="combe")
        comb_o = psum.tile([P, S], f32, tag="combo")
        for lane, comb_ps_l in ((0, comb_e), (1, comb_o)):
            for kk in range(4):
                nc.vector.tensor_scalar(m2[:], Sel[:, kk, :],
                                        par_is[:, kk:kk + 1, lane], None, AluOp.mult)
                nc.tensor.matmul(comb_ps_l[:], A_T[:, kk, :], m2[:],
                                 start=(kk == 0), stop=(kk == 3))
        ones_t = fix.tile([P, 1], bf16)
        nc.vector.memset(ones_t[:], 1.0)
        dup_ps = psum.tile([1, S], f32, tag="dup")
        for kk in range(4):
            nc.tensor.matmul(dup_ps[:], ones_t[:], LSel[:, kk, :],
                             start=(kk == 0), stop=(kk == 3))
        first_occ = fix.tile([1, S], f32)
        nc.vector.tensor_scalar(first_occ[:], dup_ps[:], 0.0, None, AluOp.is_equal)

        # ---- scatter index row: first pair-occurrence -> pair slot, else dump ----
        d1 = fix.tile([1, S], f32)
        nc.vector.tensor_scalar(d1[:], pair_f[0:1, :], -float(DUMP), None,
                                AluOp.add)
        nc.vector.scalar_tensor_tensor(out=d1[:], in0=d1[:], scalar=1.0,
                                       in1=first_occ[:], op0=AluOp.mult,
                                       op1=AluOp.mult)
        nc.vector.tensor_scalar(d1[:], d1[:], float(DUMP), None, AluOp.add)
        idxs_i = fix.tile([1, S], i16)
        nc.vector.tensor_copy(out=idxs_i[:], in_=d1[:])
        # distribute [1, 512] -> [128, 32] in CHUNKED layout: tile[p, i] =
        # row[p*32 + i]; list position j maps to source column
        # sigma(j) = (j % 16)*32 + j // 16 (adds written sigma-permuted below)
        idxs_all = fix.tile([P, 32], i16)
        # SWDGE (gpsimd) so these never stall the sync sequencer's load
        # stream; a DRAM round-trip beats SBUF hop-trees: DRAM APs can
        # broadcast, so the whole distribute is 2 triggers instead of 23
        if IDX_DRAM:
            # scalar (ACT) HWDGE ring: lower trigger latency than SWDGE
            # hops and keeps gpsimd free for the scatter
            nc.scalar.dma_start(out=scr_d, in_=idxs_i[0:1, :])
            nc.scalar.dma_start(
                out=idxs_all[:],
                in_=scr_d.rearrange("a (c i) -> (a c) i", c=16)[None, :, :]
                    .to_broadcast((8, 16, 32)))
        else:
            nc.gpsimd.dma_start(out=idxs_all[0:16, :], in_=idxs_i[0:1, :])
            nc.gpsimd.dma_start(out=idxs_all[16:32, :], in_=idxs_all[0:16, :])
            nc.gpsimd.dma_start(out=idxs_all[32:64, :], in_=idxs_all[0:32, :])
            nc.gpsimd.dma_start(out=idxs_all[64:128, :], in_=idxs_all[0:64, :])

        # ---- scatter adds: pair-packed, both lanes per entry, sigma-permuted
        # (reuses the "pair" buffer: ids_bc_i is dead once pair_f exists)
        add_pairs = work.tile([P, S, 2], bf16, tag="pair", bufs=1)
        add_v = add_pairs[:].rearrange("c (i p) d -> c p i d", p=16)
        nc.vector.tensor_scalar(add_v[:, :, :, 0],
                                comb_e[:].rearrange("c (p i) -> c p i", p=16),
                                s1[:], None, AluOp.mult)
        nc.vector.tensor_scalar(add_v[:, :, :, 1],
                                comb_o[:].rearrange("c (p i) -> c p i", p=16),
                                s1[:], None, AluOp.mult)
        nc.gpsimd.scatter_add(in_ap=pcopy[:], idxs_ap=idxs_all[:],
                              add_ap=add_pairs[:], channels=P, num_elems=NPAIR,
                              d=2, num_idxs=S)

        # ---- pass 1 tail: remaining exps (loads already in flight) ----
        for k in range(NEARLY, NT):
            off = k * VTILE
            w_k = min(VTILE, V - off)
            nc.scalar.activation(out=exp_store[:, off:off + w_k],
                                 in_=exp_store[:, off:off + w_k],
                                 func=Act.Exp, accum_out=zparts[:, k:k + 1])

        # ---- softmax scale ----
        Z = fix.tile([P, 1], f32)
        nc.vector.tensor_reduce(out=Z[:], in_=zparts[:], axis=mybir.AxisListType.X,
                                op=AluOp.add)
        invZ = fix.tile([P, 1], f32)
        nc.vector.reciprocal(out=invZ[:], in_=Z[:])
        s0 = fix.tile([P, 1], f32)
        nc.vector.tensor_mul(out=s0[:], in0=p_gen[:], in1=invZ[:])

        # ---- pass 2: fused all-bf16 blend IN PLACE over exp_store, then
        # cast-on-store straight from it. Regions are disjoint, so no
        # rotation buffers and the SWDGE queue can hold all 32 transfers.
        pcopy_flat = pcopy[:].rearrange("p a b -> p (a b)")
        # tail (smallest) tile first so the final store transfer is full-size
        for k in [NT - 1] + list(range(NT - 1)):
            off = k * VTILE
            w_k = min(VTILE, V - off)
            nc.vector.scalar_tensor_tensor(
                out=exp_store[:, off:off + w_k],
                in0=exp_store[:, off:off + w_k], scalar=s0[:],
                in1=pcopy_flat[:, off:off + w_k], op0=AluOp.mult, op1=AluOp.add)
            # SWDGE casts bf16 -> f32 on the way out
            nc.gpsimd.dma_start(out=out_d[:, off:off + w_k],
                                in_=exp_store[:, off:off + w_k])


_CACHE = {}


def _get_graph():
    if "nc" in _CACHE:
        return _CACHE["nc"]
    nc = bacc.Bacc("TRN2", target_bir_lowering=False, debug=False,
                   num_devices=NCORES)
    ids_d = nc.dram_tensor("ids", [S], i32, kind="ExternalInput").ap()
    logits_d = nc.dram_tensor("logits", [P, V], f32, kind="ExternalInput").ap()
    enc_d = nc.dram_tensor("enc", [S, D], f32, kind="ExternalInput").ap()
    dec_d = nc.dram_tensor("dec", [P, D], f32, kind="ExternalInput").ap()
    xattn_d = nc.dram_tensor("xattn", [H, P, S], f32, kind="ExternalInput").ap()
    wgw_d = nc.dram_tensor("wgw", [1, 2 * D], f32, kind="ExternalInput").ap()
    wgb_d = nc.dram_tensor("wgb", [1], f32, kind="ExternalInput").ap()
    scr_d = nc.dram_tensor("idx_scratch", [1, S], i16, kind="ExternalOutput").ap()
    out_d = nc.dram_tensor("out", [P, V], f32, kind="ExternalOutput").ap()
    with TileContext(nc) as tc:
        _body(tc, ids_d, logits_d, enc_d, dec_d, xattn_d, wgw_d, wgb_d, scr_d,
              out_d)
    nc.compile()
    _CACHE["nc"] = nc
    return nc


def _shard(inputs):
    ids = np.asarray(inputs["input_ids"])
    logits = np.asarray(inputs["logits"], dtype=np.float32)
    enc = np.asarray(inputs["encoder_hidden_states"], dtype=np.float32)
    dec = np.asarray(inputs["decoder_hidden_states"], dtype=np.float32)
    xattn = np.asarray(inputs["cross_attentions"], dtype=np.float32)
    wgw = np.asarray(inputs["W_gen_w"], dtype=np.float32)
    wgb = np.asarray(inputs["W_gen_b"], dtype=np.float32)
    in_maps = []
    for c in range(NCORES):
        b, th = c // 2, c % 2
        t0 = th * P
        in_maps.append({
            "ids": np.ascontiguousarray(ids[b]).astype(np.int32),
            "logits": np.ascontiguousarray(logits[b, t0:t0 + P, :]),
            "enc": np.ascontiguousarray(enc[b]),
            "dec": np.ascontiguousarray(dec[b, t0:t0 + P, :]),
            "xattn": np.ascontiguousarray(xattn[b, :, t0:t0 + P, :]),
            "wgw": wgw,
            "wgb": wgb,
        })
    return in_maps


def run(inputs, trace=False):
    nc = _get_graph()
    in_maps = _shard(inputs)
    res = bass_utils.run_bass_kernel_spmd(nc, in_maps,
                                          core_ids=list(range(NCORES)),
                                          trace=trace)
    out = np.empty((B, T, V), np.float32)
    for c in range(NCORES):
        b, th = c // 2, c % 2
        out[b, th * P:(th + 1) * P, :] = res.results[c]["out"]  # [P, V]
    return out, res


def kernel(**inputs):
    out, _ = run(inputs, trace=False)
    return out



# revision 65
# speedup vs baseline: 1.3943x; 1.0431x over previous
"""Copy-enhanced CodeT5 head (histogram/scatter blend) on 8 TRN2 NeuronCores.

Strategy: data-parallel over (batch, T/2) -> 8 shards of 128 decoder rows.
Each core, for its [128, V] output block:
  A_sum    = sum_h cross_attn[h]                       (DVE adds)
  p_gen    = sigmoid((A_sum @ (enc @ W1))/H + dec.W2 + b)   (PE + DVE dots + ACT)
  exp, Z   = exp(logits) streamed, row-sums via ACT accum   (pass 1)
  P_copy   = scatter-add of (1-p_gen)/H * (A_sum @ Sel) into a bf16
             pair-packed accumulator via gpsimd scatter_add; duplicate
             source ids are pre-combined with a selection-matrix matmul
             and non-first occurrences are redirected to a dump slot
             (the hardware scatter pipeline does not accumulate racing
             duplicate indices).
  out      = exp * (p_gen/Z) + P_copy                  (one fused DVE op, pass 2)

No collectives needed: every core owns a disjoint output block.
"""
import sys

sys.path.insert(0, "/opt/trn_rl_repo")

import numpy as np

import concourse.bass as bass  # noqa: F401  (registers engine classes)
import concourse.mybir as mybir
from concourse import bacc, bass_utils, library_config
from concourse.tile import TileContext
from concourse.masks import make_identity

B, S, T, D, H, V = 4, 512, 256, 1024, 16, 32105
P = 128
NCORES = 8
NPAIR = V // 2 + 2          # 16054 pair slots; pairs 0..16052 hold vocab, 16053 = dump
DUMP = NPAIR - 1
VTILE = 1024
NT = (V + VTILE - 1) // VTILE
NEARLY = 12
LIB_PRELOAD = True
IDX_DRAM = True

AluOp = mybir.AluOpType
Act = mybir.ActivationFunctionType
f32 = mybir.dt.float32
bf16 = mybir.dt.bfloat16
i32 = mybir.dt.int32
i16 = mybir.dt.int16


def _body(tc, ids_d, logits_d, enc_d, dec_d, xattn_d, wgw_d, wgb_d, scr_d,
          out_d):
    nc = tc.nc
    with tc.tile_pool(name="fix", bufs=1) as fix, \
         tc.tile_pool(name="work", bufs=4) as work, \
         tc.tile_pool(name="lpool", bufs=4) as lpool, \
         tc.tile_pool(name="psum", bufs=1, space="PSUM") as psum:

        # ---- persistent tiles ----
        exp_store = fix.tile([P, V], bf16)
        pcopy = fix.tile([P, NPAIR, 2], bf16)
        A = fix.tile([P, S], f32)

        # preload the gpsimd ucode library that scatter_add needs; the
        # ~15us load (gpsimd blocks) runs under the input-DMA phase
        if LIB_PRELOAD:
            nc.gpsimd.load_library(library_config.mlp)

        # ACT stream: hg0/hg1 head triggers, the pcopy memzero, then
        # hg2/hg3 (whose buffer-reuse waits resolve during the memzero).
        # Heads ride the scalar HWDGE ring so a blocked head trigger can
        # never head-of-line-stall the sync ring's ltile stream.
        hgs = []
        for g in range(4):
            hgs.append(work.tile([P, 4, S], f32, tag="wk", name=f"hg{g}",
                                 bufs=2))
        for g in range(2):
            nc.scalar.dma_start(out=hgs[g][:],
                                in_=xattn_d[4 * g:4 * g + 4]
                                    .rearrange("h p s -> p h s"))
        nc.scalar.memzero(pcopy[:])
        for g in range(2, 4):
            nc.scalar.dma_start(out=hgs[g][:],
                                in_=xattn_d[4 * g:4 * g + 4]
                                    .rearrange("h p s -> p h s"))

        ident = fix.tile([P, P], f32)
        make_identity(nc, ident[:])

        # ---- Q1 (sync) input DMAs, COALESCED (~0.6us sequencer cost per
        # trigger). ids first so the pair/Sel chain starts right away. No
        # trigger here may block on a consumer.
        ids_bc_i = work.tile([P, S], i32, tag="pair", bufs=1)
        nc.sync.dma_start(out=ids_bc_i[:], in_=ids_d[None, :].to_broadcast((P, S)))
        ids_col_i = fix.tile([P, 4], i32)
        nc.sync.dma_start(out=ids_col_i[:], in_=ids_d.rearrange("(c p) -> p c", p=P))
        wg = work.tile([P, 2 * D], f32, tag="wgt", bufs=1)
        nc.sync.dma_start(out=wg[:], in_=wgw_d[0:1, :].to_broadcast((P, 2 * D)))
        wb_bc = fix.tile([P, 1], f32)
        nc.sync.dma_start(out=wb_bc[:], in_=wgb_d[None, :].to_broadcast((P, 1)))
        dec_t = work.tile([P, D], f32, tag="dec", bufs=1)
        nc.sync.dma_start(out=dec_t[:], in_=dec_d[:])
        enc_g = work.tile([P, 4, D], f32, tag="enc", bufs=1)
        nc.sync.dma_start(out=enc_g[:], in_=enc_d.rearrange("(c p) d -> p c d", p=P))

        # ---- pass-1 logits stream + exps (ACT accumulator -> Z parts) ----
        zparts = fix.tile([P, NT], f32)
        ltiles = []
        for k in range(NT):
            ltile = lpool.tile([P, VTILE], f32, tag="lt", name=f"lt{k}")
            w_k = min(VTILE, V - k * VTILE)
            nc.sync.dma_start(out=ltile[:, :w_k],
                              in_=logits_d[:, k * VTILE:k * VTILE + w_k])
            ltiles.append(ltile)
        for k in range(NEARLY):
            off = k * VTILE
            w_k = min(VTILE, V - off)
            nc.scalar.activation(out=exp_store[:, off:off + w_k],
                                 in_=ltiles[k][:, :w_k],
                                 func=Act.Exp, accum_out=zparts[:, k:k + 1])

        # head sum -> A (serial: DVE keeps pace with head arrival)
        nc.vector.tensor_copy(out=A[:], in_=hgs[0][:, 0, :])
        for h in range(1, H):
            nc.vector.tensor_add(out=A[:], in0=A[:],
                                 in1=hgs[h // 4][:, h % 4, :])

        # ---- pair-level selection matrix (int compares, bf16 masks) ----
        nc.vector.tensor_scalar(ids_bc_i[:], ids_bc_i[:], 1, None,
                                AluOp.arith_shift_right)
        pair_f = work.tile([P, S], f32, tag="pair2", bufs=1)
        nc.vector.tensor_copy(out=pair_f[:], in_=ids_bc_i[:])
        parity_ci = fix.tile([P, 4], i32)
        nc.vector.tensor_scalar(parity_ci[:], ids_col_i[:], 1, None,
                                AluOp.bitwise_and)
        parity_colf = fix.tile([P, 4], f32)
        nc.vector.tensor_copy(out=parity_colf[:], in_=parity_ci[:])
        nc.vector.tensor_scalar(ids_col_i[:], ids_col_i[:], 1, None,
                                AluOp.arith_shift_right)
        pair_colf = fix.tile([P, 4], f32)
        nc.vector.tensor_copy(out=pair_colf[:], in_=ids_col_i[:])
        par_is = fix.tile([P, 4, 2], f32)
        nc.vector.tensor_scalar(par_is[:, :, 0], parity_colf[:], 0.0, None,
                                AluOp.is_equal)
        nc.vector.tensor_scalar(par_is[:, :, 1], parity_colf[:], 1.0, None,
                                AluOp.is_equal)
        Sel = fix.tile([P, 4, S], bf16)
        for kk in range(4):
            nc.vector.tensor_scalar(Sel[:, kk, :], pair_f[:],
                                    pair_colf[:, kk:kk + 1], None, AluOp.is_equal)

        # lower-triangular-masked copy (strictly s' < s) into a SEPARATE
        # tile: runs on gpsimd right after the library load, long before
        # the comb matmuls read Sel
        LSel = fix.tile([P, 4, S], bf16)
        for kk in range(4):
            nc.gpsimd.affine_select(
                out=LSel[:, kk, :], in_=Sel[:, kk, :],
                pattern=[[1, S]], compare_op=AluOp.is_ge, fill=0.0,
                base=-(kk * P) - 1, channel_multiplier=-1,
            )

        # duplicate detection: dup counts earlier same-pair occurrences
        # (emitted BEFORE the A transposes: LSel is ready much earlier)
        ones_t = fix.tile([P, 1], bf16)
        nc.vector.memset(ones_t[:], 1.0)
        dup_ps = psum.tile([1, S], f32, tag="dup")
        for kk in range(4):
            nc.tensor.matmul(dup_ps[:], ones_t[:], LSel[:, kk, :],
                             start=(kk == 0), stop=(kk == 3))
        first_occ = fix.tile([1, S], bf16)
        nc.vector.tensor_scalar(first_occ[:], dup_ps[:], 0.0, None, AluOp.is_equal)

        # ---- scatter index row, built in place over pair_f's row 0:
        # first pair-occurrence -> pair slot, else dump ----
        d1 = pair_f[0:1, :]
        nc.vector.tensor_scalar(d1, d1, -float(DUMP), None, AluOp.add)
        nc.vector.scalar_tensor_tensor(out=d1, in0=d1, scalar=1.0,
                                       in1=first_occ[:], op0=AluOp.mult,
                                       op1=AluOp.mult)
        nc.vector.tensor_scalar(d1, d1, float(DUMP), None, AluOp.add)
        idxs_i = fix.tile([1, S], i16)
        nc.vector.tensor_copy(out=idxs_i[:], in_=d1)
        # distribute [1, 512] -> [128, 32] in CHUNKED layout: tile[p, i] =
        # row[p*32 + i]; list position j maps to source column
        # sigma(j) = (j % 16)*32 + j // 16 (adds written sigma-permuted below)
        idxs_all = fix.tile([P, 32], i16)
        # DRAM round-trip: DRAM APs can broadcast, so the distribute is
        # 2 SWDGE triggers (gpsimd is free here; sync must not stall)
        nc.gpsimd.dma_start(out=scr_d, in_=idxs_i[0:1, :])
        nc.gpsimd.dma_start(
            out=idxs_all[:],
            in_=scr_d.rearrange("a (c i) -> (a c) i", c=16)[None, :, :]
                .to_broadcast((8, 16, 32)))

        # ---- A^T via PE transposes (bf16: feeds bf16 matmuls/scatter) ----
        A_T = fix.tile([P, 4, P], bf16)
        for kk in range(4):
            tps = psum.tile([P, P], f32, tag="tps", bufs=2, name=f"tps{kk}")
            nc.tensor.transpose(tps[:], A[:, kk * P:(kk + 1) * P], ident[:])
            nc.vector.tensor_copy(out=A_T[:, kk, :], in_=tps[:])

        # ---- per-lane combine matmuls (read the UNmasked Sel) ----
        m2 = fix.tile([P, S], bf16)
        comb_e = psum.tile([P, S], f32, tag="combe")
        comb_o = psum.tile([P, S], f32, tag="combo")
        for lane, comb_ps_l in ((0, comb_e), (1, comb_o)):
            for kk in range(4):
                nc.vector.tensor_scalar(m2[:], Sel[:, kk, :],
                                        par_is[:, kk:kk + 1, lane], None, AluOp.mult)
                nc.tensor.matmul(comb_ps_l[:], A_T[:, kk, :], m2[:],
                                 start=(kk == 0), stop=(kk == 3))

        # ---- p_gen (the e_sig exp lands between exp NEARLY-1 and NEARLY
        # on the in-order ACT queue) ----
        u_col = fix.tile([P, 4], f32)
        for kk in range(4):
            junk = work.tile([P, D], bf16, tag="jnk", name=f"junk{kk}", bufs=1)
            nc.vector.scalar_tensor_tensor(out=junk[:], in0=enc_g[:, kk, :],
                                           scalar=1.0,
                                           in1=wg[:, 0:D], op0=AluOp.mult,
                                           op1=AluOp.mult,
                                           accum_out=u_col[:, kk:kk + 1])
        u_colb = fix.tile([P, 4], bf16)
        nc.vector.tensor_copy(out=u_colb[:], in_=u_col[:])
        plin1_ps = psum.tile([P, 1], f32, tag="plin")
        for kk in range(4):
            nc.tensor.matmul(plin1_ps[:], A_T[:, kk, :], u_colb[:, kk:kk + 1],
                             start=(kk == 0), stop=(kk == 3))
        p_lin2 = fix.tile([P, 1], f32)
        junk2 = work.tile([P, D], bf16, tag="jnk", bufs=1)
        nc.vector.scalar_tensor_tensor(out=junk2[:], in0=dec_t[:], scalar=1.0,
                                       in1=wg[:, D:2 * D], op0=AluOp.mult,
                                       op1=AluOp.mult, accum_out=p_lin2[:])
        p_lin2b = fix.tile([P, 1], f32)
        nc.vector.tensor_add(out=p_lin2b[:], in0=p_lin2[:], in1=wb_bc[:])
        # sigmoid via the already-loaded exp table (avoids two mid-stream
        # ACT table swaps): p_gen = e/(1+e), e = exp(plin1/H + p_lin2b)
        e_sig = fix.tile([P, 1], f32)
        nc.scalar.activation(out=e_sig[:], in_=plin1_ps[:], func=Act.Exp,
                             bias=p_lin2b[:], scale=1.0 / H)
        e1 = fix.tile([P, 1], f32)
        nc.vector.tensor_scalar(e1[:], e_sig[:], 1.0, None, AluOp.add)
        e1r = fix.tile([P, 1], f32)
        nc.vector.reciprocal(out=e1r[:], in_=e1[:])
        p_gen = fix.tile([P, 1], f32)
        nc.vector.tensor_mul(out=p_gen[:], in0=e_sig[:], in1=e1r[:])
        s1 = fix.tile([P, 1], f32)
        nc.vector.tensor_scalar(s1[:], p_gen[:], -1.0 / H, 1.0 / H,
                                AluOp.mult, AluOp.add)

        # ---- scatter adds: pair-packed, both lanes per entry, sigma-
        # permuted (reuses the "pair" buffer: ids_bc_i is dead by now)
        add_pairs = work.tile([P, S, 2], bf16, tag="pair", bufs=1)
        add_v = add_pairs[:].rearrange("c (i p) d -> c p i d", p=16)
        nc.vector.tensor_scalar(add_v[:, :, :, 0],
                                comb_e[:].rearrange("c (p i) -> c p i", p=16),
                                s1[:], None, AluOp.mult)
        nc.vector.tensor_scalar(add_v[:, :, :, 1],
                                comb_o[:].rearrange("c (p i) -> c p i", p=16),
                                s1[:], None, AluOp.mult)
        nc.gpsimd.scatter_add(in_ap=pcopy[:], idxs_ap=idxs_all[:],
                              add_ap=add_pairs[:], channels=P, num_elems=NPAIR,
                              d=2, num_idxs=S)

        # ---- pass 1 tail: remaining exps (loads already in flight) ----
        for k in range(NEARLY, NT):
            off = k * VTILE
            w_k = min(VTILE, V - off)
            nc.scalar.activation(out=exp_store[:, off:off + w_k],
                                 in_=ltiles[k][:, :w_k],
                                 func=Act.Exp, accum_out=zparts[:, k:k + 1])

        # ---- softmax scale ----
        Z = fix.tile([P, 1], f32)
        nc.vector.tensor_reduce(out=Z[:], in_=zparts[:], axis=mybir.AxisListType.X,
                                op=AluOp.add)
        invZ = fix.tile([P, 1], f32)
        nc.vector.reciprocal(out=invZ[:], in_=Z[:])
        s0 = fix.tile([P, 1], f32)
        nc.vector.tensor_mul(out=s0[:], in0=p_gen[:], in1=invZ[:])

        # ---- pass 2: fused all-bf16 blend IN PLACE over exp_store, then
        # cast-on-store straight from it. Regions are disjoint, so no
        # rotation buffers and the SWDGE queue can hold all 32 transfers.
        pcopy_flat = pcopy[:].rearrange("p a b -> p (a b)")
        # tail (smallest) tile first so the final store transfer is full-size
        for k in [NT - 1] + list(range(NT - 1)):
            off = k * VTILE
            w_k = min(VTILE, V - off)
            nc.vector.scalar_tensor_tensor(
                out=exp_store[:, off:off + w_k],
                in0=exp_store[:, off:off + w_k], scalar=s0[:],
                in1=pcopy_flat[:, off:off + w_k], op0=AluOp.mult, op1=AluOp.add)
            # SWDGE casts bf16 -> f32 on the way out
            nc.gpsimd.dma_start(out=out_d[:, off:off + w_k],
                                in_=exp_store[:, off:off + w_k])


_CACHE = {}


def _get_graph():
    if "nc" in _CACHE:
        return _CACHE["nc"]
    nc = bacc.Bacc("TRN2", target_bir_lowering=False, debug=False,
                   num_devices=NCORES)
    ids_d = nc.dram_tensor("ids", [S], i32, kind="ExternalInput").ap()
    logits_d = nc.dram_tensor("logits", [P, V], f32, kind="ExternalInput").ap()
    enc_d = nc.dram_tensor("enc", [S, D], f32, kind="ExternalInput").ap()
    dec_d = nc.dram_tensor("dec", [P, D], f32, kind="ExternalInput").ap()
    xattn_d = nc.dram_tensor("xattn", [H, P, S], f32, kind="ExternalInput").ap()
    wgw_d = nc.dram_tensor("wgw", [1, 2 * D], f32, kind="ExternalInput").ap()
    wgb_d = nc.dram_tensor("wgb", [1], f32, kind="ExternalInput").ap()
    scr_d = nc.dram_tensor("idx_scratch", [1, S], i16, kind="ExternalOutput").ap()
    out_d = nc.dram_tensor("out", [P, V], f32, kind="ExternalOutput").ap()
    with TileContext(nc) as tc:
        _body(tc, ids_d, logits_d, enc_d, dec_d, xattn_d, wgw_d, wgb_d, scr_d,
              out_d)
    nc.compile()
    _CACHE["nc"] = nc
    return nc


def _shard(inputs):
    ids = np.asarray(inputs["input_ids"])
    logits = np.asarray(inputs["logits"], dtype=np.float32)
    enc = np.asarray(inputs["encoder_hidden_states"], dtype=np.float32)
    dec = np.asarray(inputs["decoder_hidden_states"], dtype=np.float32)
    xattn = np.asarray(inputs["cross_attentions"], dtype=np.float32)
    wgw = np.asarray(inputs["W_gen_w"], dtype=np.float32)
    wgb = np.asarray(inputs["W_gen_b"], dtype=np.float32)
    in_maps = []
    for c in range(NCORES):
        b, th = c // 2, c % 2
        t0 = th * P
        in_maps.append({
            "ids": np.ascontiguousarray(ids[b]).astype(np.int32),
            "logits": np.ascontiguousarray(logits[b, t0:t0 + P, :]),
            "enc": np.ascontiguousarray(enc[b]),
            "dec": np.ascontiguousarray(dec[b, t0:t0 + P, :]),
            "xattn": np.ascontiguousarray(xattn[b, :, t0:t0 + P, :]),
            "wgw": wgw,
            "wgb": wgb,
        })
    return in_maps


def run(inputs, trace=False):
    nc = _get_graph()
    in_maps = _shard(inputs)
    res = bass_utils.run_bass_kernel_spmd(nc, in_maps,
                                          core_ids=list(range(NCORES)),
                                          trace=trace)
    out = np.empty((B, T, V), np.float32)
    for c in range(NCORES):
        b, th = c // 2, c % 2
        out[b, th * P:(th + 1) * P, :] = res.results[c]["out"]  # [P, V]
    return out, res


def kernel(**inputs):
    out, _ = run(inputs, trace=False)
    return out



# revision 69
# speedup vs baseline: 1.4100x; 1.0113x over previous
"""Copy-enhanced CodeT5 head (histogram/scatter blend) on 8 TRN2 NeuronCores.

Strategy: data-parallel over (batch, T/2) -> 8 shards of 128 decoder rows.
Each core, for its [128, V] output block:
  A_sum    = sum_h cross_attn[h]                       (DVE adds)
  p_gen    = sigmoid((A_sum @ (enc @ W1))/H + dec.W2 + b)   (PE + DVE dots + ACT)
  exp, Z   = exp(logits) streamed, row-sums via ACT accum   (pass 1)
  P_copy   = scatter-add of (1-p_gen)/H * (A_sum @ Sel) into a bf16
             pair-packed accumulator via gpsimd scatter_add; duplicate
             source ids are pre-combined with a selection-matrix matmul
             and non-first occurrences are redirected to a dump slot
             (the hardware scatter pipeline does not accumulate racing
             duplicate indices).
  out      = exp * (p_gen/Z) + P_copy                  (one fused DVE op, pass 2)

No collectives needed: every core owns a disjoint output block.
"""
import sys

sys.path.insert(0, "/opt/trn_rl_repo")

import numpy as np

import concourse.bass as bass  # noqa: F401  (registers engine classes)
import concourse.mybir as mybir
from concourse import bacc, bass_utils, library_config
from concourse.tile import TileContext
from concourse.masks import make_identity

B, S, T, D, H, V = 4, 512, 256, 1024, 16, 32105
P = 128
NCORES = 8
NPAIR = V // 2 + 2          # 16054 pair slots; pairs 0..16052 hold vocab, 16053 = dump
DUMP = NPAIR - 1
VTILE = 1024
NT = (V + VTILE - 1) // VTILE
NEARLY = 7
LIB_PRELOAD = True
IDX_DRAM = True

AluOp = mybir.AluOpType
Act = mybir.ActivationFunctionType
f32 = mybir.dt.float32
bf16 = mybir.dt.bfloat16
i32 = mybir.dt.int32
i16 = mybir.dt.int16


def _body(tc, ids_d, logits_d, enc_d, dec_d, xattn_d, wgw_d, wgb_d, scr_d,
          out_d):
    nc = tc.nc
    with tc.tile_pool(name="fix", bufs=1) as fix, \
         tc.tile_pool(name="work", bufs=4) as work, \
         tc.tile_pool(name="lpool", bufs=4) as lpool, \
         tc.tile_pool(name="psum", bufs=1, space="PSUM") as psum:

        # ---- persistent tiles ----
        exp_store = fix.tile([P, V], bf16)
        pcopy = fix.tile([P, NPAIR, 2], bf16)

        # ---- heads: SWDGE cast-on-load to bf16 (Q0 ring, concurrent with
        # Q1), all four groups resident so no trigger ever blocks. Summed
        # on the otherwise-idle TensorEngine below.
        hgs = []
        for g in range(4):
            hg = work.tile([P, 4, S], bf16, tag="wk", name=f"hg{g}", bufs=4)
            nc.gpsimd.dma_start(out=hg[:],
                                in_=xattn_d[4 * g:4 * g + 4]
                                    .rearrange("h p s -> p h s"))
            hgs.append(hg)

        # preload the gpsimd ucode library that scatter_add needs; the
        # ~15us load (gpsimd blocks) runs under the input-DMA phase
        if LIB_PRELOAD:
            nc.gpsimd.load_library(library_config.mlp)

        nc.scalar.memzero(pcopy[:])

        ident = fix.tile([P, P], bf16)
        make_identity(nc, ident[:])

        # ---- Q1 (sync) input DMAs, COALESCED (~0.6us sequencer cost per
        # trigger). ids first so the pair/Sel chain starts right away. No
        # trigger here may block on a consumer.
        ids_bc_i = work.tile([P, S], i32, tag="pair", bufs=1)
        nc.sync.dma_start(out=ids_bc_i[:], in_=ids_d[None, :].to_broadcast((P, S)))
        ids_col_i = fix.tile([P, 4], i32)
        nc.sync.dma_start(out=ids_col_i[:], in_=ids_d.rearrange("(c p) -> p c", p=P))
        wg = work.tile([P, 2 * D], f32, tag="wgt", bufs=1)
        nc.sync.dma_start(out=wg[:], in_=wgw_d[0:1, :].to_broadcast((P, 2 * D)))
        wb_bc = fix.tile([P, 1], f32)
        nc.sync.dma_start(out=wb_bc[:], in_=wgb_d[None, :].to_broadcast((P, 1)))
        dec_t = work.tile([P, D], f32, tag="dec", bufs=1)
        nc.sync.dma_start(out=dec_t[:], in_=dec_d[:])
        enc_g = work.tile([P, 4, D], f32, tag="enc", bufs=1)
        nc.sync.dma_start(out=enc_g[:], in_=enc_d.rearrange("(c p) d -> p c d", p=P))

        # ---- pass-1 logits stream + exps (ACT accumulator -> Z parts) ----
        zparts = fix.tile([P, NT], f32)
        ltiles = []
        for k in range(NT):
            ltile = lpool.tile([P, VTILE], f32, tag="lt", name=f"lt{k}")
            w_k = min(VTILE, V - k * VTILE)
            nc.sync.dma_start(out=ltile[:, :w_k],
                              in_=logits_d[:, k * VTILE:k * VTILE + w_k])
            ltiles.append(ltile)
        for k in range(NEARLY):
            off = k * VTILE
            w_k = min(VTILE, V - off)
            nc.scalar.activation(out=exp_store[:, off:off + w_k],
                                 in_=ltiles[k][:, :w_k],
                                 func=Act.Exp, accum_out=zparts[:, k:k + 1])

        # head sum on the TensorEngine: identity-matmul accumulation in
        # PSUM (stationary ident weights, ~0.4us per head, PE is idle)
        A_ps = psum.tile([P, S], f32, tag="aps")
        for h in range(H):
            nc.tensor.matmul(A_ps[:], ident[:], hgs[h // 4][:, h % 4, :],
                             start=(h == 0), stop=(h == H - 1))
        A = fix.tile([P, S], bf16)
        nc.vector.tensor_copy(out=A[:], in_=A_ps[:])

        # ---- pair-level selection matrix (int compares, bf16 masks) ----
        nc.vector.tensor_scalar(ids_bc_i[:], ids_bc_i[:], 1, None,
                                AluOp.arith_shift_right)
        pair_f = work.tile([P, S], f32, tag="pair2", bufs=1)
        nc.vector.tensor_copy(out=pair_f[:], in_=ids_bc_i[:])
        parity_ci = fix.tile([P, 4], i32)
        nc.vector.tensor_scalar(parity_ci[:], ids_col_i[:], 1, None,
                                AluOp.bitwise_and)
        parity_colf = fix.tile([P, 4], f32)
        nc.vector.tensor_copy(out=parity_colf[:], in_=parity_ci[:])
        nc.vector.tensor_scalar(ids_col_i[:], ids_col_i[:], 1, None,
                                AluOp.arith_shift_right)
        pair_colf = fix.tile([P, 4], f32)
        nc.vector.tensor_copy(out=pair_colf[:], in_=ids_col_i[:])
        par_is = fix.tile([P, 4, 2], f32)
        nc.vector.tensor_scalar(par_is[:, :, 0], parity_colf[:], 0.0, None,
                                AluOp.is_equal)
        nc.vector.tensor_scalar(par_is[:, :, 1], parity_colf[:], 1.0, None,
                                AluOp.is_equal)
        Sel = fix.tile([P, 4, S], bf16)
        for kk in range(4):
            nc.vector.tensor_scalar(Sel[:, kk, :], pair_f[:],
                                    pair_colf[:, kk:kk + 1], None, AluOp.is_equal)

        # lower-triangular-masked copy (strictly s' < s) into a SEPARATE
        # tile: runs on gpsimd right after the library load, long before
        # the comb matmuls read Sel
        LSel = fix.tile([P, 4, S], bf16)
        for kk in range(4):
            nc.gpsimd.affine_select(
                out=LSel[:, kk, :], in_=Sel[:, kk, :],
                pattern=[[1, S]], compare_op=AluOp.is_ge, fill=0.0,
                base=-(kk * P) - 1, channel_multiplier=-1,
            )

        # duplicate detection: dup counts earlier same-pair occurrences
        # (emitted BEFORE the A transposes: LSel is ready much earlier)
        ones_t = fix.tile([P, 1], bf16)
        nc.vector.memset(ones_t[:], 1.0)
        dup_ps = psum.tile([1, S], f32, tag="dup")
        for kk in range(4):
            nc.tensor.matmul(dup_ps[:], ones_t[:], LSel[:, kk, :],
                             start=(kk == 0), stop=(kk == 3))
        first_occ = fix.tile([1, S], bf16)
        nc.vector.tensor_scalar(first_occ[:], dup_ps[:], 0.0, None, AluOp.is_equal)

        # ---- scatter index row, built in place over pair_f's row 0:
        # first pair-occurrence -> pair slot, else dump ----
        d1 = pair_f[0:1, :]
        nc.vector.tensor_scalar(d1, d1, -float(DUMP), None, AluOp.add)
        nc.vector.scalar_tensor_tensor(out=d1, in0=d1, scalar=1.0,
                                       in1=first_occ[:], op0=AluOp.mult,
                                       op1=AluOp.mult)
        nc.vector.tensor_scalar(d1, d1, float(DUMP), None, AluOp.add)
        idxs_i = fix.tile([1, S], i16)
        nc.vector.tensor_copy(out=idxs_i[:], in_=d1)
        # distribute [1, 512] -> [128, 32] in CHUNKED layout: tile[p, i] =
        # row[p*32 + i]; list position j maps to source column
        # sigma(j) = (j % 16)*32 + j // 16 (adds written sigma-permuted below)
        idxs_all = fix.tile([P, 32], i16)
        # DRAM round-trip: DRAM APs can broadcast, so the distribute is
        # 2 SWDGE triggers (gpsimd is free here; sync must not stall)
        nc.gpsimd.dma_start(out=scr_d, in_=idxs_i[0:1, :])
        nc.gpsimd.dma_start(
            out=idxs_all[:],
            in_=scr_d.rearrange("a (c i) -> (a c) i", c=16)[None, :, :]
                .to_broadcast((8, 16, 32)))

        # ---- A^T via PE transposes (bf16: feeds bf16 matmuls/scatter) ----
        A_T = fix.tile([P, 4, P], bf16)
        for kk in range(4):
            tps = psum.tile([P, P], bf16, tag="tps", bufs=2, name=f"tps{kk}")
            nc.tensor.transpose(tps[:], A[:, kk * P:(kk + 1) * P], ident[:])
            nc.vector.tensor_copy(out=A_T[:, kk, :], in_=tps[:])

        # ---- per-lane combine matmuls (read the UNmasked Sel) ----
        m2 = fix.tile([P, S], bf16)
        comb_e = psum.tile([P, S], f32, tag="combe")
        comb_o = psum.tile([P, S], f32, tag="combo")
        for lane, comb_ps_l in ((0, comb_e), (1, comb_o)):
            for kk in range(4):
                nc.vector.tensor_scalar(m2[:], Sel[:, kk, :],
                                        par_is[:, kk:kk + 1, lane], None, AluOp.mult)
                nc.tensor.matmul(comb_ps_l[:], A_T[:, kk, :], m2[:],
                                 start=(kk == 0), stop=(kk == 3))

        # ---- p_gen (the e_sig exp lands between exp NEARLY-1 and NEARLY
        # on the in-order ACT queue) ----
        u_col = fix.tile([P, 4], f32)
        for kk in range(4):
            junk = work.tile([P, D], bf16, tag="jnk", name=f"junk{kk}", bufs=1)
            nc.vector.scalar_tensor_tensor(out=junk[:], in0=enc_g[:, kk, :],
                                           scalar=1.0,
                                           in1=wg[:, 0:D], op0=AluOp.mult,
                                           op1=AluOp.mult,
                                           accum_out=u_col[:, kk:kk + 1])
        u_colb = fix.tile([P, 4], bf16)
        nc.vector.tensor_copy(out=u_colb[:], in_=u_col[:])
        plin1_ps = psum.tile([P, 1], f32, tag="plin")
        for kk in range(4):
            nc.tensor.matmul(plin1_ps[:], A_T[:, kk, :], u_colb[:, kk:kk + 1],
                             start=(kk == 0), stop=(kk == 3))
        p_lin2 = fix.tile([P, 1], f32)
        junk2 = work.tile([P, D], bf16, tag="jnk", bufs=1)
        nc.vector.scalar_tensor_tensor(out=junk2[:], in0=dec_t[:], scalar=1.0,
                                       in1=wg[:, D:2 * D], op0=AluOp.mult,
                                       op1=AluOp.mult, accum_out=p_lin2[:])
        p_lin2b = fix.tile([P, 1], f32)
        nc.vector.tensor_add(out=p_lin2b[:], in0=p_lin2[:], in1=wb_bc[:])
        # sigmoid via the already-loaded exp table (avoids two mid-stream
        # ACT table swaps): p_gen = e/(1+e), e = exp(plin1/H + p_lin2b)
        e_sig = fix.tile([P, 1], f32)
        nc.scalar.activation(out=e_sig[:], in_=plin1_ps[:], func=Act.Exp,
                             bias=p_lin2b[:], scale=1.0 / H)
        e1 = fix.tile([P, 1], f32)
        nc.vector.tensor_scalar(e1[:], e_sig[:], 1.0, None, AluOp.add)
        e1r = fix.tile([P, 1], f32)
        nc.vector.reciprocal(out=e1r[:], in_=e1[:])
        p_gen = fix.tile([P, 1], f32)
        nc.vector.tensor_mul(out=p_gen[:], in0=e_sig[:], in1=e1r[:])
        s1 = fix.tile([P, 1], f32)
        nc.vector.tensor_scalar(s1[:], p_gen[:], -1.0 / H, 1.0 / H,
                                AluOp.mult, AluOp.add)

        # ---- scatter adds: pair-packed, both lanes per entry, sigma-
        # permuted (reuses the "pair" buffer: ids_bc_i is dead by now)
        add_pairs = work.tile([P, S, 2], bf16, tag="pair", bufs=1)
        add_v = add_pairs[:].rearrange("c (i p) d -> c p i d", p=16)
        nc.vector.tensor_scalar(add_v[:, :, :, 0],
                                comb_e[:].rearrange("c (p i) -> c p i", p=16),
                                s1[:], None, AluOp.mult)
        nc.vector.tensor_scalar(add_v[:, :, :, 1],
                                comb_o[:].rearrange("c (p i) -> c p i", p=16),
                                s1[:], None, AluOp.mult)
        nc.gpsimd.scatter_add(in_ap=pcopy[:], idxs_ap=idxs_all[:],
                              add_ap=add_pairs[:], channels=P, num_elems=NPAIR,
                              d=2, num_idxs=S)

        # ---- pass 1 tail: remaining exps (loads already in flight) ----
        for k in range(NEARLY, NT):
            off = k * VTILE
            w_k = min(VTILE, V - off)
            nc.scalar.activation(out=exp_store[:, off:off + w_k],
                                 in_=ltiles[k][:, :w_k],
                                 func=Act.Exp, accum_out=zparts[:, k:k + 1])

        # ---- softmax scale ----
        Z = fix.tile([P, 1], f32)
        nc.vector.tensor_reduce(out=Z[:], in_=zparts[:], axis=mybir.AxisListType.X,
                                op=AluOp.add)
        invZ = fix.tile([P, 1], f32)
        nc.vector.reciprocal(out=invZ[:], in_=Z[:])
        s0 = fix.tile([P, 1], f32)
        nc.vector.tensor_mul(out=s0[:], in0=p_gen[:], in1=invZ[:])

        # ---- pass 2: fused all-bf16 blend IN PLACE over exp_store, then
        # cast-on-store straight from it. Regions are disjoint, so no
        # rotation buffers and the SWDGE queue can hold all 32 transfers.
        pcopy_flat = pcopy[:].rearrange("p a b -> p (a b)")
        # tail (smallest) tile first so the final store transfer is full-size
        for k in [NT - 1] + list(range(NT - 1)):
            off = k * VTILE
            w_k = min(VTILE, V - off)
            nc.vector.scalar_tensor_tensor(
                out=exp_store[:, off:off + w_k],
                in0=exp_store[:, off:off + w_k], scalar=s0[:],
                in1=pcopy_flat[:, off:off + w_k], op0=AluOp.mult, op1=AluOp.add)
            # SWDGE casts bf16 -> f32 on the way out
            nc.gpsimd.dma_start(out=out_d[:, off:off + w_k],
                                in_=exp_store[:, off:off + w_k])


_CACHE = {}


def _get_graph():
    if "nc" in _CACHE:
        return _CACHE["nc"]
    nc = bacc.Bacc("TRN2", target_bir_lowering=False, debug=False,
                   num_devices=NCORES)
    ids_d = nc.dram_tensor("ids", [S], i32, kind="ExternalInput").ap()
    logits_d = nc.dram_tensor("logits", [P, V], f32, kind="ExternalInput").ap()
    enc_d = nc.dram_tensor("enc", [S, D], f32, kind="ExternalInput").ap()
    dec_d = nc.dram_tensor("dec", [P, D], f32, kind="ExternalInput").ap()
    xattn_d = nc.dram_tensor("xattn", [H, P, S], f32, kind="ExternalInput").ap()
    wgw_d = nc.dram_tensor("wgw", [1, 2 * D], f32, kind="ExternalInput").ap()
    wgb_d = nc.dram_tensor("wgb", [1], f32, kind="ExternalInput").ap()
    scr_d = nc.dram_tensor("idx_scratch", [1, S], i16, kind="ExternalOutput").ap()
    out_d = nc.dram_tensor("out", [P, V], f32, kind="ExternalOutput").ap()
    with TileContext(nc) as tc:
        _body(tc, ids_d, logits_d, enc_d, dec_d, xattn_d, wgw_d, wgb_d, scr_d,
              out_d)
    nc.compile()
    _CACHE["nc"] = nc
    return nc


def _shard(inputs):
    ids = np.asarray(inputs["input_ids"])
    logits = np.asarray(inputs["logits"], dtype=np.float32)
    enc = np.asarray(inputs["encoder_hidden_states"], dtype=np.float32)
    dec = np.asarray(inputs["decoder_hidden_states"], dtype=np.float32)
    xattn = np.asarray(inputs["cross_attentions"], dtype=np.float32)
    wgw = np.asarray(inputs["W_gen_w"], dtype=np.float32)
    wgb = np.asarray(inputs["W_gen_b"], dtype=np.float32)
    in_maps = []
    for c in range(NCORES):
        b, th = c // 2, c % 2
        t0 = th * P
        in_maps.append({
            "ids": np.ascontiguousarray(ids[b]).astype(np.int32),
            "logits": np.ascontiguousarray(logits[b, t0:t0 + P, :]),
            "enc": np.ascontiguousarray(enc[b]),
            "dec": np.ascontiguousarray(dec[b, t0:t0 + P, :]),
            "xattn": np.ascontiguousarray(xattn[b, :, t0:t0 + P, :]),
            "wgw": wgw,
            "wgb": wgb,
        })
    return in_maps


def run(inputs, trace=False):
    nc = _get_graph()
    in_maps = _shard(inputs)
    res = bass_utils.run_bass_kernel_spmd(nc, in_maps,
                                          core_ids=list(range(NCORES)),
                                          trace=trace)
    out = np.empty((B, T, V), np.float32)
    for c in range(NCORES):
        b, th = c // 2, c % 2
        out[b, th * P:(th + 1) * P, :] = res.results[c]["out"]  # [P, V]
    return out, res


def kernel(**inputs):
    out, _ = run(inputs, trace=False)
    return out

